# revision 6
# baseline (speedup 1.0000x reference)
"""Trainium2 Bass kernel for sparse-depth k-NN (nn_Dist).

For every pixel q of a 96x128 grid, find the 4 nearest valid pixels
(S > 0.001) by Euclidean distance, with jax.lax.top_k tie-breaking
(equal distance -> lowest linear index first).

Device algorithm (8 NeuronCores, SPMD over query rows, 1536 queries/core):
  score(q, c) = -|q - c|^2 + const(q) = 2*qx*cx + 2*qy*cy - (cx^2 + cy^2)
computed as a K=3 matmul on the TensorEngine into PSUM (every product and
partial sum is an integer below 2^24, so fp32 arithmetic is exact), then
the ScalarEngine copies scores to SBUF and the VectorEngine MAX8 / MAX_INDEX
instructions produce the top-8 scores and their candidate positions per
query (HW tie-break: first occurrence = lowest index, matching top_k).
Host maps positions -> pixel indices/coords.

Raw Bass (explicit semaphores): the Tile scheduler emits multiple embedded
sync-waits on Matmult instructions, which walrus codegen rejects ("Too many
sync wait commands" - the PE LDWEIGHTS struct holds one). Standalone
wait_ge sequencer ops sidestep that.
"""

import numpy as np

H, W = 96, 128
N = H * W                    # 12288 queries
N_NEIGHBORS = 4
V_THRESH = 0.001
N_CORES = 8
QPC = N // N_CORES           # 1536 queries per core
P = 128                      # partitions
TILES = QPC // P             # 12 query tiles per core
PAD_SCORE = -1.0e9           # score for padded (invalid) candidate slots
MAX_C = 4096                 # PSUM free-dim capacity (fp32)
PSUM_WORDS = 4096            # fp32 words per partition in all 8 banks

_module_cache = {}
LAST_RESULTS = None  # BassKernelResults of the most recent device run


def _build_module(C):
    """Raw-Bass module for C candidate columns (compile-time const)."""
    import concourse.bass as bass
    import concourse.mybir as mybir

    f32 = mybir.dt.float32
    u32 = mybir.dt.uint32

    # PSUM slot ring: slot stride is a power-of-two word count >= C so every
    # matmul chunk lands bank-aligned.
    slot_words = 1024
    while slot_words < C:
        slot_words *= 2
    n_slots = max(1, PSUM_WORDS // slot_words)

    nc = bass.Bass()
    AB = nc.dram_tensor("AB", [3, QPC + C], f32, kind="ExternalInput")
    OUT = nc.dram_tensor("OUT", [P, TILES * 8], u32, kind="ExternalOutput")

    with (
        nc.sbuf_tensor("ab_t", [3, QPC + C], f32) as ab_t,
        nc.sbuf_tensor("sc_all", [P, TILES * C], f32) as sc_all,
        nc.sbuf_tensor("mx_all", [P, TILES * 8], f32) as mx_all,
        nc.sbuf_tensor("ix_all", [P, TILES * 8], u32) as ix_all,
        nc.psum_tensor("ps", [P, PSUM_WORDS], f32) as ps,
        nc.semaphore("dma_in") as dma_in,
        nc.semaphore("pe_sem") as pe_sem,
        nc.semaphore("act_sem") as act_sem,
        nc.semaphore("dve_sem") as dve_sem,
        nc.semaphore("dma_out") as dma_out,
        nc.Block() as block,
    ):

        @block.sync
        def _(sync):
            sync.dma_start(ab_t[:], AB[:]).then_inc(dma_in, 16)
            sync.wait_ge(dve_sem, 2 * TILES)
            sync.dma_start(OUT[:], ix_all[:]).then_inc(dma_out, 16)
            sync.wait_ge(dma_out, 16)

        @block.tensor
        def _(tensor):
            tensor.wait_ge(dma_in, 16)
            for i in range(TILES):
                if i >= n_slots:
                    # PSUM slot (i % n_slots) must be drained by the ACT copy
                    tensor.wait_ge(act_sem, i - n_slots + 1)
                base = (i % n_slots) * slot_words
                lhsT = ab_t[:, i * P:(i + 1) * P]
                last = None
                for j0 in range(0, C, 512):
                    j1 = min(j0 + 512, C)
                    last = tensor.matmul(
                        ps[:, base + j0:base + j1],
                        lhsT,
                        ab_t[:, QPC + j0:QPC + j1],
                    )
                last.then_inc(pe_sem)

        @block.scalar
        def _(scalar):
            for i in range(TILES):
                scalar.wait_ge(pe_sem, i + 1)
                base = (i % n_slots) * slot_words
                scalar.copy(
                    sc_all[:, i * C:(i + 1) * C],
                    ps[:, base:base + C],
                ).then_inc(act_sem)

        @block.vector
        def _(vector):
            for i in range(TILES):
                vector.wait_ge(act_sem, i + 1)
                sc_i = sc_all[:, i * C:(i + 1) * C]
                vector.max(
                    out=mx_all[:, i * 8:(i + 1) * 8], in_=sc_i
                ).then_inc(dve_sem)
                vector.wait_ge(dve_sem, 2 * i + 1)
                vector.max_index(
                    out=ix_all[:, i * 8:(i + 1) * 8],
                    in_max=mx_all[:, i * 8:(i + 1) * 8],
                    in_values=sc_i,
                ).then_inc(dve_sem)

    return nc


def _get_module(C):
    if C not in _module_cache:
        _module_cache[C] = _build_module(C)
    return _module_cache[C]


def _run_device(ABmat, C):
    """ABmat [3, QPC+C] per core -> positions (N, 8) int64."""
    from concourse.bass_utils import run_bass_kernel_spmd

    nc = _get_module(C)
    in_maps = [{"AB": ab} for ab in ABmat]
    res = run_bass_kernel_spmd(nc, in_maps, core_ids=list(range(N_CORES)))
    global LAST_RESULTS
    LAST_RESULTS = res
    outs = []
    for r in res.results:
        o = r["OUT"].reshape(P, TILES, 8)          # [p, tile, rank]
        outs.append(o.transpose(1, 0, 2).reshape(QPC, 8))
    return np.concatenate(outs, axis=0).astype(np.int64)


def _host_fallback(flat, valid_idx):
    """Exact numpy replication of the reference for degenerate inputs."""
    q = np.arange(N)
    qx = (q % W).astype(np.float32)
    qy = (q // W).astype(np.float32)
    cx = (valid_idx % W).astype(np.float32)
    cy = (valid_idx // W).astype(np.float32)
    pos4 = np.empty((N, N_NEIGHBORS), np.int64)
    chunk = 512
    for s in range(0, N, chunk):
        e = min(s + chunk, N)
        dx = qx[s:e, None] - cx[None, :]
        dy = qy[s:e, None] - cy[None, :]
        sc = np.full((e - s, N), -np.inf, np.float32)
        sc[:, valid_idx] = -(dx * dx + dy * dy)
        order = np.argsort(-sc, axis=1, kind="stable")
        pos4[s:e] = order[:, :N_NEIGHBORS]
    return pos4  # already pixel indices (full-N score rows)


def kernel(S):
    S = np.asarray(S)
    flat = S.reshape(-1).astype(np.float32)
    valid_idx = np.flatnonzero(flat > V_THRESH)
    n_valid = int(valid_idx.size)

    if n_valid < 8 or n_valid > MAX_C:
        args_nq = _host_fallback(flat, valid_idx)          # (N, 4) pixel idx
    else:
        C = max(P, ((n_valid + P - 1) // P) * P)
        cx = (valid_idx % W).astype(np.float32)
        cy = (valid_idx // W).astype(np.float32)

        q = np.arange(N)
        qx = (q % W).astype(np.float32)
        qy = (q // W).astype(np.float32)

        ABmat = []
        for c in range(N_CORES):
            ab = np.zeros((3, QPC + C), np.float32)
            sl = slice(c * QPC, (c + 1) * QPC)
            ab[0, :QPC] = 2.0 * qx[sl]
            ab[1, :QPC] = 2.0 * qy[sl]
            ab[2, :QPC] = 1.0
            ab[0, QPC:QPC + n_valid] = cx
            ab[1, QPC:QPC + n_valid] = cy
            ab[2, QPC:QPC + n_valid] = -(cx * cx + cy * cy)
            ab[2, QPC + n_valid:] = PAD_SCORE
            ABmat.append(ab)

        positions = _run_device(ABmat, C)                  # (N, 8)
        args_nq = valid_idx[positions[:, :N_NEIGHBORS]]    # (N, 4) pixel idx

    args = args_nq.T.astype(np.int32)[None]                # (1, 4, N)
    ipc = np.empty((1, 2, N_NEIGHBORS, N), np.float32)
    ipc[0, 0] = (args[0] % W).astype(np.float32)
    ipc[0, 1] = (args[0] // W).astype(np.float32)
    return ipc, args


# revision 7
# speedup vs baseline: 1.8080x; 1.8080x over previous
"""Trainium2 Bass kernel for sparse-depth k-NN (nn_Dist).

For every pixel q of a 96x128 grid, find the 4 nearest valid pixels
(S > 0.001) by Euclidean distance, with jax.lax.top_k tie-breaking
(equal distance -> lowest linear index first).

Device algorithm (8 NeuronCores, SPMD over query rows, 1536 queries/core):
the TensorEngine computes, for each query q and candidate c,

    key(q, c) = C * (2*qx*cx + 2*qy*cy - cx^2 - cy^2) - idx_c
              = C * (-|q-c|^2 + qx^2 + qy^2) - idx_c

as a bf16 matmul accumulated in fp32 PSUM. Every factor is an integer
exactly representable in bf16 (split into high/low parts) and every
product / partial sum stays an exact fp32 integer (|.| bounds checked
below), so keys are EXACT. Keys order candidates per query by
(distance asc, index asc) — exactly jax.lax.top_k order — and are unique,
so the VectorEngine MAX8 instruction alone (top-8 values per partition)
yields the top-4; the host decodes idx = (-key) mod C. MAX8 reads PSUM
directly; no intermediate copies.

For C > 640 the scaled key would overflow the 2^24 exact-integer range,
so a fallback variant computes the unscaled score (K=4 bf16 matmul) and
uses MAX8 + MAX_INDEX (HW tie-break = first occurrence = lowest index,
verified exact vs top_k on HW).

Raw Bass with explicit semaphores: the Tile scheduler emits multiple
embedded sync-waits on Matmult instructions, which walrus codegen rejects
(the PE LDWEIGHTS struct holds one); standalone wait_ge ops avoid that.
"""

import numpy as np

H, W = 96, 128
N = H * W                    # 12288 queries
N_NEIGHBORS = 4
V_THRESH = 0.001
N_CORES = 8
QPC = N // N_CORES           # 1536 queries per core
P = 128                      # partitions
TILES = QPC // P             # 12 query tiles per core
MAX_C_KEYED = 640            # C*(25154+1) must stay < 2^24 for keyed variant
MAX_C = 4096                 # PSUM free-dim capacity (fp32)
PSUM_WORDS = 4096            # fp32 words per partition in all 8 banks

_module_cache = {}
LAST_RESULTS = None  # BassKernelResults of the most recent device run


def _build_module(C, keyed):
    """Raw-Bass module for C candidate columns.

    keyed=True : K=7 matmul of index-encoded keys, MAX8 only, fp32 out.
    keyed=False: K=4 matmul of plain scores, MAX8 + MAX_INDEX, uint32 out.
    """
    import concourse.bass as bass
    import concourse.mybir as mybir

    f32 = mybir.dt.float32
    u32 = mybir.dt.uint32
    bf16 = mybir.dt.bfloat16
    K = 7 if keyed else 4

    slot_words = 1024
    while slot_words < C:
        slot_words *= 2
    n_slots = max(1, PSUM_WORDS // slot_words)

    nc = bass.Bass(enable_partition_id=False)
    AB = nc.dram_tensor("AB", [K, QPC + C], bf16, kind="ExternalInput")
    out_dt = f32 if keyed else u32
    OUT = nc.dram_tensor("OUT", [P, TILES * 8], out_dt, kind="ExternalOutput")

    with (
        nc.sbuf_tensor("ab_t", [K, QPC + C], bf16) as ab_t,
        nc.sbuf_tensor("mx_all", [P, TILES * 8], f32) as mx_all,
        nc.psum_tensor("ps", [P, PSUM_WORDS], f32) as ps,
        nc.semaphore("dma_in") as dma_in,
        nc.semaphore("pe_sem") as pe_sem,
        nc.semaphore("dve_sem") as dve_sem,
        nc.semaphore("dma_out") as dma_out,
    ):
        if keyed:
            _emit(nc, bass, C, slot_words, n_slots, AB, OUT, ab_t, mx_all,
                  None, ps, dma_in, pe_sem, dve_sem, dma_out, keyed=True)
        else:
            with nc.sbuf_tensor("ix_all", [P, TILES * 8], u32) as ix_all:
                _emit(nc, bass, C, slot_words, n_slots, AB, OUT, ab_t, mx_all,
                      ix_all, ps, dma_in, pe_sem, dve_sem, dma_out, keyed=False)
    return nc


def _emit(nc, bass, C, slot_words, n_slots, AB, OUT, ab_t, mx_all, ix_all,
          ps, dma_in, pe_sem, dve_sem, dma_out, keyed):
    dve_per_tile = 1 if keyed else 2
    out_sb = mx_all if keyed else ix_all

    with nc.Block() as block:

        @block.sync
        def _(sync):
            sync.dma_start(ab_t[:], AB[:]).then_inc(dma_in, 16)
            sync.wait_ge(dve_sem, dve_per_tile * TILES)
            sync.dma_start(OUT[:], out_sb[:]).then_inc(dma_out, 16)
            sync.wait_ge(dma_out, 16)

        @block.tensor
        def _(tensor):
            tensor.wait_ge(dma_in, 16)
            for i in range(TILES):
                if i >= n_slots:
                    # PSUM slot (i % n_slots) must be drained by the DVE
                    tensor.wait_ge(
                        dve_sem, dve_per_tile * (i - n_slots + 1))
                base = (i % n_slots) * slot_words
                lhsT = ab_t[:, i * P:(i + 1) * P]
                last = None
                for j0 in range(0, C, 512):
                    j1 = min(j0 + 512, C)
                    last = tensor.matmul(
                        ps[:, base + j0:base + j1],
                        lhsT,
                        ab_t[:, QPC + j0:QPC + j1],
                    )
                last.then_inc(pe_sem)

        @block.vector
        def _(vector):
            for i in range(TILES):
                vector.wait_ge(pe_sem, i + 1)
                base = (i % n_slots) * slot_words
                sc_i = ps[:, base:base + C]
                vector.max(
                    out=mx_all[:, i * 8:(i + 1) * 8], in_=sc_i
                ).then_inc(dve_sem)
                if not keyed:
                    vector.wait_ge(dve_sem, 2 * i + 1)
                    vector.max_index(
                        out=ix_all[:, i * 8:(i + 1) * 8],
                        in_max=mx_all[:, i * 8:(i + 1) * 8],
                        in_values=sc_i,
                    ).then_inc(dve_sem)


def _get_module(C, keyed):
    key = (C, keyed)
    if key not in _module_cache:
        _module_cache[key] = _build_module(C, keyed)
    return _module_cache[key]


def _run_device(ABmat, C, keyed):
    """ABmat: list of [K, QPC+C] bf16 per core -> (N, 8) out values."""
    from concourse.bass_utils import run_bass_kernel_spmd

    nc = _get_module(C, keyed)
    in_maps = [{"AB": ab} for ab in ABmat]
    res = run_bass_kernel_spmd(nc, in_maps, core_ids=list(range(N_CORES)))
    global LAST_RESULTS
    LAST_RESULTS = res
    outs = []
    for r in res.results:
        o = r["OUT"].reshape(P, TILES, 8)          # [p, tile, rank]
        outs.append(o.transpose(1, 0, 2).reshape(QPC, 8))
    return np.concatenate(outs, axis=0)


def _query_features(keyed, C):
    """Per-query lhsT rows [K, N] as float (bf16-exact integer values)."""
    q = np.arange(N)
    qx = (q % W).astype(np.float64)
    qy = (q // W).astype(np.float64)
    if keyed:
        # key = 2C*qx*cx + 2C*qy*cy - C*(cx^2+cy^2) - idx, C = 640:
        # 2C*qx = 1280*qx split as 20480*(qx>>4) + 1280*(qx&15).
        # K order chosen so partial sums are exact fp32 integers under
        # either PE accumulation direction (see analysis in module doc).
        rows = [
            20480.0 * np.floor(qx / 16),   # * cx
            np.full(N, -65536.0),          # * v2
            20480.0 * np.floor(qy / 16),   # * cy
            1280.0 * (qx % 16),            # * cx
            1280.0 * (qy % 16),            # * cy
            np.full(N, -256.0),            # * v1
            np.full(N, -1.0),              # * v0
        ]
    else:
        rows = [
            2.0 * qx,                      # * cx
            2.0 * qy,                      # * cy
            np.full(N, -256.0),            # * w1
            np.full(N, -1.0),              # * w0
        ]
    return np.stack(rows)                  # [K, N]


def _cand_features(keyed, C, valid_idx):
    """Per-candidate rhs rows [K, C] incl. padding columns."""
    n_valid = valid_idx.size
    cx = (valid_idx % W).astype(np.float64)
    cy = (valid_idx // W).astype(np.float64)
    Bm = np.zeros((7 if keyed else 4, C), np.float64)
    if keyed:
        v = 640.0 * (cx * cx + cy * cy) + np.arange(n_valid, dtype=np.float64)
        assert v.max(initial=0) < 2 ** 24
        Bm[0, :n_valid] = cx
        Bm[1, :n_valid] = np.floor(v / 65536)
        Bm[2, :n_valid] = cy
        Bm[3, :n_valid] = cx
        Bm[4, :n_valid] = cy
        Bm[5, :n_valid] = np.floor(v / 256) % 256
        Bm[6, :n_valid] = v % 256
        # padding: key = -(65536+256+1)*255 = -16777215 < any real key
        Bm[1, n_valid:] = 255.0
        Bm[5, n_valid:] = 255.0
        Bm[6, n_valid:] = 255.0
    else:
        w = cx * cx + cy * cy              # <= 25154
        Bm[0, :n_valid] = cx
        Bm[1, :n_valid] = cy
        Bm[2, :n_valid] = np.floor(w / 256)
        Bm[3, :n_valid] = w % 256
        # padding: score = -(256*255+255) = -65535 < any real score min
        # (real score = -d2 + qx^2+qy^2 >= -25154)
        Bm[2, n_valid:] = 255.0
        Bm[3, n_valid:] = 255.0
    return Bm


def _host_fallback(flat, valid_idx):
    """Exact numpy replication of the reference for degenerate inputs."""
    q = np.arange(N)
    qx = (q % W).astype(np.float32)
    qy = (q // W).astype(np.float32)
    cx = (valid_idx % W).astype(np.float32)
    cy = (valid_idx // W).astype(np.float32)
    pos4 = np.empty((N, N_NEIGHBORS), np.int64)
    chunk = 512
    for s in range(0, N, chunk):
        e = min(s + chunk, N)
        dx = qx[s:e, None] - cx[None, :]
        dy = qy[s:e, None] - cy[None, :]
        sc = np.full((e - s, N), -np.inf, np.float32)
        sc[:, valid_idx] = -(dx * dx + dy * dy)
        order = np.argsort(-sc, axis=1, kind="stable")
        pos4[s:e] = order[:, :N_NEIGHBORS]
    return pos4  # already pixel indices (full-N score rows)


def kernel(S):
    import ml_dtypes

    S = np.asarray(S)
    flat = S.reshape(-1).astype(np.float32)
    valid_idx = np.flatnonzero(flat > V_THRESH)
    n_valid = int(valid_idx.size)

    if n_valid < 8 or n_valid > MAX_C:
        args_nq = _host_fallback(flat, valid_idx)          # (N, 4) pixel idx
    else:
        C = max(P, ((n_valid + P - 1) // P) * P)
        keyed = C <= MAX_C_KEYED

        Arows = _query_features(keyed, C)                  # [K, N]
        Brows = _cand_features(keyed, C, valid_idx)        # [K, C]
        Kdim = Arows.shape[0]
        ABmat = []
        for c in range(N_CORES):
            ab = np.empty((Kdim, QPC + C), np.float64)
            ab[:, :QPC] = Arows[:, c * QPC:(c + 1) * QPC]
            ab[:, QPC:] = Brows
            ABmat.append(np.ascontiguousarray(
                ab.astype(ml_dtypes.bfloat16)))

        out = _run_device(ABmat, C, keyed)                 # (N, 8)
        if keyed:
            keys = out[:, :N_NEIGHBORS].astype(np.float64)
            positions = np.mod(-keys, 640.0).astype(np.int64)
        else:
            positions = out[:, :N_NEIGHBORS].astype(np.int64)
        args_nq = valid_idx[positions]                     # (N, 4) pixel idx

    args = args_nq.T.astype(np.int32)[None]                # (1, 4, N)
    ipc = np.empty((1, 2, N_NEIGHBORS, N), np.float32)
    ipc[0, 0] = (args[0] % W).astype(np.float32)
    ipc[0, 1] = (args[0] // W).astype(np.float32)
    return ipc, args


# revision 9
# speedup vs baseline: 2.1596x; 1.1945x over previous
"""Trainium2 Bass kernel for sparse-depth k-NN (nn_Dist).

For every pixel q of a 96x128 grid, find the 4 nearest valid pixels
(S > 0.001) by Euclidean distance, with jax.lax.top_k tie-breaking
(equal distance -> lowest linear index first).

Device algorithm (8 NeuronCores, SPMD over query rows, 1536 queries/core,
12 tiles of 128 queries = one pixel row per tile): the TensorEngine
computes, for each query q and candidate c,

    key(q, c) = 640 * (2*qx*cx + 2*qy*cy - cx^2 - cy^2) - idx_c
              = 640 * (-|q-c|^2 + qx^2 + qy^2) - idx_c

as a bf16 matmul (K=7: every factor split into bf16-exact integer parts)
accumulated in fp32 PSUM. Every product / partial sum stays an exact fp32
integer under either PE accumulation direction, so keys are EXACT. Keys
order candidates per query by (distance asc, index asc) — exactly
jax.lax.top_k order — and are unique, so the VectorEngine MAX8 instruction
alone (top-8 values per partition, read straight from PSUM) yields the
top-4; the host decodes idx = (-key) mod 640.

Candidate pruning: each tile is one pixel row y0. A cell-ring bound (8x8
cells; the smallest ring around a cell holding >= 4 valid pixels bounds
every in-cell query's 4th-NN distance) gives a per-row radius R(y0); any
candidate with |cy - y0| > R cannot be in that row's top-4, so each tile
only scores its y-band of candidates (typically ~halving the MAX8 stream).

For candidate counts > 640 the scaled key would overflow the 2^24
exact-integer range, so a fallback variant computes the unscaled score
(K=4 bf16 matmul) and uses MAX8 + MAX_INDEX (HW tie-break = first
occurrence = lowest index, verified exact vs top_k on HW).

Raw Bass with explicit semaphores: the Tile scheduler emits multiple
embedded sync-waits on Matmult instructions, which walrus codegen rejects
(the PE LDWEIGHTS struct holds one); standalone wait_ge ops avoid that.
"""

import numpy as np

H, W = 96, 128
N = H * W                    # 12288 queries
N_NEIGHBORS = 4
V_THRESH = 0.001
N_CORES = 8
QPC = N // N_CORES           # 1536 queries per core
P = 128                      # partitions
TILES = QPC // P             # 12 query tiles (pixel rows) per core
KEY_M = 640.0                # key multiplier; KEY_M*(25154+1) < 2^24
MAX_C_KEYED = 640            # idx < KEY_M and exactness both need C <= 640
MAX_C = 4096                 # PSUM free-dim capacity (fp32)
PSUM_WORDS = 4096            # fp32 words per partition in all 8 banks

_module_cache = {}
LAST_RESULTS = None  # BassKernelResults of the most recent device run


def _build_module(C, keyed, per_tile_b):
    """Raw-Bass module for C candidate columns.

    keyed=True : K=7 matmul of index-encoded keys, MAX8 only, fp32 out.
    keyed=False: K=4 matmul of plain scores, MAX8 + MAX_INDEX, uint32 out.
    per_tile_b : each tile has its own C candidate columns (y-band pruning).
    """
    import concourse.bass as bass
    import concourse.mybir as mybir

    f32 = mybir.dt.float32
    u32 = mybir.dt.uint32
    bf16 = mybir.dt.bfloat16
    K = 7 if keyed else 4
    nb = TILES if per_tile_b else 1

    slot_words = 512
    while slot_words < C:
        slot_words *= 2
    n_slots = max(1, PSUM_WORDS // slot_words)

    nc = bass.Bass(enable_partition_id=False)
    AB = nc.dram_tensor("AB", [K, QPC + nb * C], bf16, kind="ExternalInput")
    out_dt = f32 if keyed else u32
    OUT = nc.dram_tensor("OUT", [P, TILES * 8], out_dt, kind="ExternalOutput")

    with (
        nc.sbuf_tensor("ab_t", [K, QPC + nb * C], bf16) as ab_t,
        nc.sbuf_tensor("mx_all", [P, TILES * 8], f32) as mx_all,
        nc.psum_tensor("ps", [P, PSUM_WORDS], f32) as ps,
        nc.semaphore("dma_in") as dma_in,
        nc.semaphore("pe_sem") as pe_sem,
        nc.semaphore("dve_sem") as dve_sem,
        nc.semaphore("dma_out") as dma_out,
    ):
        if keyed:
            _emit(nc, C, nb, slot_words, n_slots, AB, OUT, ab_t, mx_all,
                  None, ps, dma_in, pe_sem, dve_sem, dma_out, keyed=True)
        else:
            with nc.sbuf_tensor("ix_all", [P, TILES * 8], u32) as ix_all:
                _emit(nc, C, nb, slot_words, n_slots, AB, OUT, ab_t, mx_all,
                      ix_all, ps, dma_in, pe_sem, dve_sem, dma_out,
                      keyed=False)
    return nc


def _emit(nc, C, nb, slot_words, n_slots, AB, OUT, ab_t, mx_all, ix_all,
          ps, dma_in, pe_sem, dve_sem, dma_out, keyed):
    dve_per_tile = 1 if keyed else 2
    out_sb = mx_all if keyed else ix_all
    half = (TILES // 2) * 8

    with nc.Block() as block:

        @block.sync
        def _(sync):
            sync.dma_start(ab_t[:], AB[:]).then_inc(dma_in, 16)
            # overlap: ship the first half of the results mid-stream
            sync.wait_ge(dve_sem, dve_per_tile * (TILES // 2))
            sync.dma_start(OUT[:, :half], out_sb[:, :half]).then_inc(
                dma_out, 16)
            sync.wait_ge(dve_sem, dve_per_tile * TILES)
            sync.dma_start(OUT[:, half:], out_sb[:, half:]).then_inc(
                dma_out, 16)
            sync.wait_ge(dma_out, 32)

        @block.tensor
        def _(tensor):
            tensor.wait_ge(dma_in, 16)
            for i in range(TILES):
                if i >= n_slots:
                    # PSUM slot (i % n_slots) must be drained by the DVE
                    tensor.wait_ge(
                        dve_sem, dve_per_tile * (i - n_slots + 1))
                base = (i % n_slots) * slot_words
                boff = QPC + (i % nb) * C if nb > 1 else QPC
                lhsT = ab_t[:, i * P:(i + 1) * P]
                last = None
                for j0 in range(0, C, 512):
                    j1 = min(j0 + 512, C)
                    last = tensor.matmul(
                        ps[:, base + j0:base + j1],
                        lhsT,
                        ab_t[:, boff + j0:boff + j1],
                    )
                last.then_inc(pe_sem)

        @block.vector
        def _(vector):
            for i in range(TILES):
                vector.wait_ge(pe_sem, i + 1)
                base = (i % n_slots) * slot_words
                sc_i = ps[:, base:base + C]
                vector.max(
                    out=mx_all[:, i * 8:(i + 1) * 8], in_=sc_i
                ).then_inc(dve_sem)
                if not keyed:
                    vector.wait_ge(dve_sem, 2 * i + 1)
                    vector.max_index(
                        out=ix_all[:, i * 8:(i + 1) * 8],
                        in_max=mx_all[:, i * 8:(i + 1) * 8],
                        in_values=sc_i,
                    ).then_inc(dve_sem)


def _get_module(C, keyed, per_tile_b):
    key = (C, keyed, per_tile_b)
    if key not in _module_cache:
        _module_cache[key] = _build_module(C, keyed, per_tile_b)
    return _module_cache[key]


def _run_device(ABmat, C, keyed, per_tile_b):
    """ABmat: list of [K, cols] bf16 per core -> (N, 8) out values."""
    from concourse.bass_utils import run_bass_kernel_spmd

    nc = _get_module(C, keyed, per_tile_b)
    in_maps = [{"AB": ab} for ab in ABmat]
    res = run_bass_kernel_spmd(nc, in_maps, core_ids=list(range(N_CORES)))
    global LAST_RESULTS
    LAST_RESULTS = res
    outs = []
    for r in res.results:
        o = r["OUT"].reshape(P, TILES, 8)          # [p, tile, rank]
        outs.append(o.transpose(1, 0, 2).reshape(QPC, 8))
    return np.concatenate(outs, axis=0)


def _query_features(keyed):
    """Per-query lhsT rows [K, N] as float (bf16-exact integer values)."""
    q = np.arange(N)
    qx = (q % W).astype(np.float64)
    qy = (q // W).astype(np.float64)
    if keyed:
        # 2*KEY_M*qx = 1280*qx split as 20480*(qx>>4) + 1280*(qx&15).
        # K order chosen so partial sums stay exact fp32 integers under
        # either PE accumulation direction.
        rows = [
            20480.0 * np.floor(qx / 16),   # * cx
            np.full(N, -65536.0),          # * v2
            20480.0 * np.floor(qy / 16),   # * cy
            1280.0 * (qx % 16),            # * cx
            1280.0 * (qy % 16),            # * cy
            np.full(N, -256.0),            # * v1
            np.full(N, -1.0),              # * v0
        ]
    else:
        rows = [
            2.0 * qx,                      # * cx
            2.0 * qy,                      # * cy
            np.full(N, -256.0),            # * w1
            np.full(N, -1.0),              # * w0
        ]
    return np.stack(rows)                  # [K, N]


def _cand_features(keyed, C, cand_idx):
    """Per-candidate rhs rows [K, C] incl. padding columns.

    cand_idx: pixel indices of this block's candidates (ascending)."""
    n = cand_idx.size
    cx = (cand_idx % W).astype(np.float64)
    cy = (cand_idx // W).astype(np.float64)
    Bm = np.zeros((7 if keyed else 4, C), np.float64)
    if keyed:
        v = KEY_M * (cx * cx + cy * cy) + np.arange(n, dtype=np.float64)
        assert n <= KEY_M and v.max(initial=0) < 2 ** 24
        Bm[0, :n] = cx
        Bm[1, :n] = np.floor(v / 65536)
        Bm[2, :n] = cy
        Bm[3, :n] = cx
        Bm[4, :n] = cy
        Bm[5, :n] = np.floor(v / 256) % 256
        Bm[6, :n] = v % 256
        # padding: key = -(65536+256+1)*255 = -16777215 < any real key
        Bm[1, n:] = 255.0
        Bm[5, n:] = 255.0
        Bm[6, n:] = 255.0
    else:
        w = cx * cx + cy * cy              # <= 25154
        Bm[0, :n] = cx
        Bm[1, :n] = cy
        Bm[2, :n] = np.floor(w / 256)
        Bm[3, :n] = w % 256
        # padding: score = -65535 < real score min (-25154)
        Bm[2, n:] = 255.0
        Bm[3, n:] = 255.0
    return Bm


def _row_radius(valid_idx):
    """Per pixel row y0: radius R such that every query in row y0 has its
    4 nearest valid pixels within |cy - y0| <= R. Cell-ring bound on an
    8x8-pixel cell grid (exact upper bound on the 4th-NN distance)."""
    cx = valid_idx % W
    cy = valid_idx // W
    gj, gi = H // 8, W // 8                       # 12 x 16 cells
    cnt = np.zeros((gj, gi), np.int64)
    np.add.at(cnt, (cy // 8, cx // 8), 1)
    # ring sums via 2D prefix
    pre = np.zeros((gj + 1, gi + 1), np.int64)
    pre[1:, 1:] = cnt.cumsum(0).cumsum(1)

    def box(j0, j1, i0, i1):                      # inclusive cell box sum
        j0 = max(j0, 0); i0 = max(i0, 0)
        j1 = min(j1, gj - 1); i1 = min(i1, gi - 1)
        return pre[j1 + 1, i1 + 1] - pre[j0, i1 + 1] \
            - pre[j1 + 1, i0] + pre[j0, i0]

    bound = np.zeros((gj, gi))
    for j in range(gj):
        for i in range(gi):
            for r in range(max(gj, gi)):
                if box(j - r, j + r, i - r, i + r) >= N_NEIGHBORS:
                    bound[j, i] = np.sqrt(2.0) * (8 * (r + 1) - 1)
                    break
            else:
                bound[j, i] = np.sqrt(2.0) * 8 * max(gj, gi)
    row_bound = bound.max(axis=1)                 # per cell row
    return np.ceil(row_bound[np.arange(H) // 8]).astype(np.int64)  # per row


def _host_fallback(flat, valid_idx):
    """Exact numpy replication of the reference for degenerate inputs."""
    q = np.arange(N)
    qx = (q % W).astype(np.float32)
    qy = (q // W).astype(np.float32)
    cx = (valid_idx % W).astype(np.float32)
    cy = (valid_idx // W).astype(np.float32)
    pos4 = np.empty((N, N_NEIGHBORS), np.int64)
    chunk = 512
    for s in range(0, N, chunk):
        e = min(s + chunk, N)
        dx = qx[s:e, None] - cx[None, :]
        dy = qy[s:e, None] - cy[None, :]
        sc = np.full((e - s, N), -np.inf, np.float32)
        sc[:, valid_idx] = -(dx * dx + dy * dy)
        order = np.argsort(-sc, axis=1, kind="stable")
        pos4[s:e] = order[:, :N_NEIGHBORS]
    return pos4  # already pixel indices (full-N score rows)


def _pack_ab(Arows, Bblocks, C):
    """Assemble per-core AB matrices. Bblocks: [N_CORES][nb] of [K, C]."""
    import ml_dtypes

    Kdim = Arows.shape[0]
    ABmat = []
    for c in range(N_CORES):
        nb = len(Bblocks[c])
        ab = np.empty((Kdim, QPC + nb * C), np.float64)
        ab[:, :QPC] = Arows[:, c * QPC:(c + 1) * QPC]
        for i, blk in enumerate(Bblocks[c]):
            ab[:, QPC + i * C:QPC + (i + 1) * C] = blk
        ABmat.append(np.ascontiguousarray(ab.astype(ml_dtypes.bfloat16)))
    return ABmat


def kernel(S):
    S = np.asarray(S)
    flat = S.reshape(-1).astype(np.float32)
    valid_idx = np.flatnonzero(flat > V_THRESH)
    n_valid = int(valid_idx.size)

    if n_valid < 8 or n_valid > MAX_C:
        args_nq = _host_fallback(flat, valid_idx)
    else:
        cy = valid_idx // W
        R = _row_radius(valid_idx)                         # (H,)
        bands = [valid_idx[(cy >= y0 - R[y0]) & (cy <= y0 + R[y0])]
                 for y0 in range(H)]
        c_band = max(b.size for b in bands)
        C_tile = max(64, ((c_band + 63) // 64) * 64)

        if C_tile <= MAX_C_KEYED and C_tile < n_valid:
            # banded keyed path: per-tile candidate y-bands
            Arows = _query_features(True)
            Bblocks = [[_cand_features(True, C_tile, bands[12 * c + i])
                        for i in range(TILES)] for c in range(N_CORES)]
            out = _run_device(_pack_ab(Arows, Bblocks, C_tile),
                              C_tile, True, True)
            keys = out[:, :N_NEIGHBORS].astype(np.float64)
            pos = np.mod(-keys, KEY_M).astype(np.int64)    # (N, 4) local
            band_arr = np.zeros((H, C_tile), np.int64)
            for y0 in range(H):
                band_arr[y0, :bands[y0].size] = bands[y0]
            args_nq = band_arr[(np.arange(N) // W)[:, None], pos]
        else:
            # shared full candidate set
            C = max(P, ((n_valid + P - 1) // P) * P)
            keyed = C <= MAX_C_KEYED
            Arows = _query_features(keyed)
            Bblock = _cand_features(keyed, C, valid_idx)
            out = _run_device(_pack_ab(Arows, [[Bblock]] * N_CORES, C),
                              C, keyed, False)
            if keyed:
                keys = out[:, :N_NEIGHBORS].astype(np.float64)
                pos = np.mod(-keys, KEY_M).astype(np.int64)
            else:
                pos = out[:, :N_NEIGHBORS].astype(np.int64)
            args_nq = valid_idx[pos]

    args = args_nq.T.astype(np.int32)[None]                # (1, 4, N)
    ipc = np.empty((1, 2, N_NEIGHBORS, N), np.float32)
    ipc[0, 0] = (args[0] % W).astype(np.float32)
    ipc[0, 1] = (args[0] // W).astype(np.float32)
    return ipc, args


# revision 11
# speedup vs baseline: 2.2669x; 1.0497x over previous
"""Trainium2 Bass kernel for sparse-depth k-NN (nn_Dist).

For every pixel q of a 96x128 grid, find the 4 nearest valid pixels
(S > 0.001) by Euclidean distance, with jax.lax.top_k tie-breaking
(equal distance -> lowest linear index first).

Device algorithm (8 NeuronCores, SPMD over query rows, 1536 queries/core,
12 tiles of 128 queries = one pixel row per tile): the TensorEngine
computes, for each query q and candidate c,

    key(q, c) = 640 * (2*qx*cx + 2*qy*cy - cx^2 - cy^2) - idx_c
              = 640 * (-|q-c|^2 + qx^2 + qy^2) - idx_c

as a bf16 matmul (K=7: every factor split into bf16-exact integer parts)
accumulated in fp32 PSUM. Every product / partial sum stays an exact fp32
integer under either PE accumulation direction, so keys are EXACT. Keys
order candidates per query by (distance asc, index asc) — exactly
jax.lax.top_k order — and are unique, so the VectorEngine MAX8 instruction
alone (top-8 values per partition, read straight from PSUM) yields the
top-4; the host decodes idx = (-key) mod 640.

Candidate pruning: each tile is one pixel row y0. A cell-ring bound (8x8
cells; the smallest ring around a cell holding >= 4 valid pixels bounds
every in-cell query's 4th-NN distance) gives a per-row radius R(y0); any
candidate with |cy - y0| > R cannot be in that row's top-4, so each tile
only scores its y-band of candidates (typically ~halving the MAX8 stream).

For candidate counts > 640 the scaled key would overflow the 2^24
exact-integer range, so a fallback variant computes the unscaled score
(K=4 bf16 matmul) and uses MAX8 + MAX_INDEX (HW tie-break = first
occurrence = lowest index, verified exact vs top_k on HW).

Raw Bass with explicit semaphores: the Tile scheduler emits multiple
embedded sync-waits on Matmult instructions, which walrus codegen rejects
(the PE LDWEIGHTS struct holds one); standalone wait_ge ops avoid that.
"""

import numpy as np

H, W = 96, 128
N = H * W                    # 12288 queries
N_NEIGHBORS = 4
V_THRESH = 0.001
N_CORES = 8
QPC = N // N_CORES           # 1536 queries per core
P = 128                      # partitions
TILES = QPC // P             # 12 query tiles (pixel rows) per core
KEY_M = 640.0                # key multiplier; KEY_M*(25154+1) < 2^24
MAX_C_KEYED = 640            # idx < KEY_M and exactness both need C <= 640
MAX_C = 4096                 # PSUM free-dim capacity (fp32)
PSUM_WORDS = 4096            # fp32 words per partition in all 8 banks

_module_cache = {}
LAST_RESULTS = None  # BassKernelResults of the most recent device run


def _build_module(C, keyed, per_tile_b):
    """Raw-Bass module for C candidate columns.

    keyed=True : K=7 matmul of index-encoded keys, MAX8 only, fp32 out.
    keyed=False: K=4 matmul of plain scores, MAX8 + MAX_INDEX, uint32 out.
    per_tile_b : each tile has its own C candidate columns (y-band pruning).
    """
    import concourse.bass as bass
    import concourse.mybir as mybir

    f32 = mybir.dt.float32
    u32 = mybir.dt.uint32
    bf16 = mybir.dt.bfloat16
    K = 7 if keyed else 4
    nb = TILES if per_tile_b else 1

    slot_words = 512
    while slot_words < C:
        slot_words *= 2
    n_slots = max(1, PSUM_WORDS // slot_words)

    nc = bass.Bass(enable_partition_id=False, enable_asserts=False,
                   monotonic_sem_count=0)
    AB = nc.dram_tensor("AB", [K, QPC + nb * C], bf16, kind="ExternalInput")
    out_dt = f32 if keyed else u32
    OUT = nc.dram_tensor("OUT", [P, TILES * 8], out_dt, kind="ExternalOutput")

    with (
        nc.sbuf_tensor("ab_t", [K, QPC + nb * C], bf16) as ab_t,
        nc.sbuf_tensor("mx_all", [P, TILES * 8], f32) as mx_all,
        nc.psum_tensor("ps", [P, PSUM_WORDS], f32) as ps,
        nc.semaphore("dma_in") as dma_in,
        nc.semaphore("pe_sem") as pe_sem,
        nc.semaphore("dve_sem") as dve_sem,
        nc.semaphore("dma_out") as dma_out,
    ):
        if keyed:
            _emit(nc, C, nb, slot_words, n_slots, AB, OUT, ab_t, mx_all,
                  None, ps, dma_in, pe_sem, dve_sem, dma_out, keyed=True)
        else:
            with nc.sbuf_tensor("ix_all", [P, TILES * 8], u32) as ix_all:
                _emit(nc, C, nb, slot_words, n_slots, AB, OUT, ab_t, mx_all,
                      ix_all, ps, dma_in, pe_sem, dve_sem, dma_out,
                      keyed=False)
    return nc


def _emit(nc, C, nb, slot_words, n_slots, AB, OUT, ab_t, mx_all, ix_all,
          ps, dma_in, pe_sem, dve_sem, dma_out, keyed):
    dve_per_tile = 1 if keyed else 2
    out_sb = mx_all if keyed else ix_all
    half = (TILES // 2) * 8

    with nc.Block() as block:

        @block.sync
        def _(sync):
            sync.dma_start(ab_t[:], AB[:]).then_inc(dma_in, 16)
            # overlap: ship the first half of the results mid-stream
            sync.wait_ge(dve_sem, dve_per_tile * (TILES // 2))
            sync.dma_start(OUT[:, :half], out_sb[:, :half]).then_inc(
                dma_out, 16)
            sync.wait_ge(dve_sem, dve_per_tile * TILES)
            sync.dma_start(OUT[:, half:], out_sb[:, half:]).then_inc(
                dma_out, 16)
            sync.wait_ge(dma_out, 32)

        @block.tensor
        def _(tensor):
            tensor.wait_ge(dma_in, 16)
            for i in range(TILES):
                if i >= n_slots:
                    # PSUM slot (i % n_slots) must be drained by the DVE
                    tensor.wait_ge(
                        dve_sem, dve_per_tile * (i - n_slots + 1))
                base = (i % n_slots) * slot_words
                boff = QPC + (i % nb) * C if nb > 1 else QPC
                lhsT = ab_t[:, i * P:(i + 1) * P]
                last = None
                for j0 in range(0, C, 512):
                    j1 = min(j0 + 512, C)
                    last = tensor.matmul(
                        ps[:, base + j0:base + j1],
                        lhsT,
                        ab_t[:, boff + j0:boff + j1],
                    )
                last.then_inc(pe_sem)

        @block.vector
        def _(vector):
            for i in range(TILES):
                vector.wait_ge(pe_sem, i + 1)
                base = (i % n_slots) * slot_words
                sc_i = ps[:, base:base + C]
                vector.max(
                    out=mx_all[:, i * 8:(i + 1) * 8], in_=sc_i
                ).then_inc(dve_sem)
                if not keyed:
                    vector.wait_ge(dve_sem, 2 * i + 1)
                    vector.max_index(
                        out=ix_all[:, i * 8:(i + 1) * 8],
                        in_max=mx_all[:, i * 8:(i + 1) * 8],
                        in_values=sc_i,
                    ).then_inc(dve_sem)


def _get_module(C, keyed, per_tile_b):
    key = (C, keyed, per_tile_b)
    if key not in _module_cache:
        _module_cache[key] = _build_module(C, keyed, per_tile_b)
    return _module_cache[key]


def _run_device(ABmat, C, keyed, per_tile_b):
    """ABmat: list of [K, cols] bf16 per core -> (N, 8) out values."""
    from concourse.bass_utils import run_bass_kernel_spmd

    nc = _get_module(C, keyed, per_tile_b)
    in_maps = [{"AB": ab} for ab in ABmat]
    res = run_bass_kernel_spmd(nc, in_maps, core_ids=list(range(N_CORES)))
    global LAST_RESULTS
    LAST_RESULTS = res
    outs = []
    for r in res.results:
        o = r["OUT"].reshape(P, TILES, 8)          # [p, tile, rank]
        outs.append(o.transpose(1, 0, 2).reshape(QPC, 8))
    return np.concatenate(outs, axis=0)


def _query_features(keyed):
    """Per-query lhsT rows [K, N] as float (bf16-exact integer values)."""
    q = np.arange(N)
    qx = (q % W).astype(np.float64)
    qy = (q // W).astype(np.float64)
    if keyed:
        # 2*KEY_M*qx = 1280*qx split as 20480*(qx>>4) + 1280*(qx&15).
        # K order chosen so partial sums stay exact fp32 integers under
        # either PE accumulation direction.
        rows = [
            20480.0 * np.floor(qx / 16),   # * cx
            np.full(N, -65536.0),          # * v2
            20480.0 * np.floor(qy / 16),   # * cy
            1280.0 * (qx % 16),            # * cx
            1280.0 * (qy % 16),            # * cy
            np.full(N, -256.0),            # * v1
            np.full(N, -1.0),              # * v0
        ]
    else:
        rows = [
            2.0 * qx,                      # * cx
            2.0 * qy,                      # * cy
            np.full(N, -256.0),            # * w1
            np.full(N, -1.0),              # * w0
        ]
    return np.stack(rows)                  # [K, N]


def _cand_features(keyed, C, cand_idx):
    """Per-candidate rhs rows [K, C] incl. padding columns.

    cand_idx: pixel indices of this block's candidates (ascending)."""
    n = cand_idx.size
    cx = (cand_idx % W).astype(np.float64)
    cy = (cand_idx // W).astype(np.float64)
    Bm = np.zeros((7 if keyed else 4, C), np.float64)
    if keyed:
        v = KEY_M * (cx * cx + cy * cy) + np.arange(n, dtype=np.float64)
        assert n <= KEY_M and v.max(initial=0) < 2 ** 24
        Bm[0, :n] = cx
        Bm[1, :n] = np.floor(v / 65536)
        Bm[2, :n] = cy
        Bm[3, :n] = cx
        Bm[4, :n] = cy
        Bm[5, :n] = np.floor(v / 256) % 256
        Bm[6, :n] = v % 256
        # padding: key = -(65536+256+1)*255 = -16777215 < any real key
        Bm[1, n:] = 255.0
        Bm[5, n:] = 255.0
        Bm[6, n:] = 255.0
    else:
        w = cx * cx + cy * cy              # <= 25154
        Bm[0, :n] = cx
        Bm[1, :n] = cy
        Bm[2, :n] = np.floor(w / 256)
        Bm[3, :n] = w % 256
        # padding: score = -65535 < real score min (-25154)
        Bm[2, n:] = 255.0
        Bm[3, n:] = 255.0
    return Bm


def _row_radius_cells(valid_idx, cs):
    """Cell-ring bound with cell size cs: per pixel row y0, a radius R such
    that every query in row y0 has its 4 nearest valid pixels within
    |cy - y0| <= R (exact upper bound on the 4th-NN distance)."""
    cx = valid_idx % W
    cy = valid_idx // W
    gj, gi = -(-H // cs), -(-W // cs)
    cnt = np.zeros((gj, gi), np.int64)
    np.add.at(cnt, (cy // cs, cx // cs), 1)
    pre = np.zeros((gj + 1, gi + 1), np.int64)
    pre[1:, 1:] = cnt.cumsum(0).cumsum(1)

    def box(j0, j1, i0, i1):                      # inclusive cell box sum
        j0 = max(j0, 0); i0 = max(i0, 0)
        j1 = min(j1, gj - 1); i1 = min(i1, gi - 1)
        return pre[j1 + 1, i1 + 1] - pre[j0, i1 + 1] \
            - pre[j1 + 1, i0] + pre[j0, i0]

    bound = np.zeros((gj, gi))
    for j in range(gj):
        for i in range(gi):
            for r in range(max(gj, gi) + 1):
                if box(j - r, j + r, i - r, i + r) >= N_NEIGHBORS:
                    bound[j, i] = np.sqrt(2.0) * (cs * (r + 1) - 1)
                    break
            else:
                bound[j, i] = np.sqrt(2.0) * cs * (max(gj, gi) + 1)
    row_bound = bound.max(axis=1)                 # per cell row
    return np.ceil(row_bound[np.arange(H) // cs]).astype(np.int64)


def _row_radius(valid_idx):
    return np.minimum(_row_radius_cells(valid_idx, 8),
                      _row_radius_cells(valid_idx, 4))


def _host_fallback(flat, valid_idx):
    """Exact numpy replication of the reference for degenerate inputs."""
    q = np.arange(N)
    qx = (q % W).astype(np.float32)
    qy = (q // W).astype(np.float32)
    cx = (valid_idx % W).astype(np.float32)
    cy = (valid_idx // W).astype(np.float32)
    pos4 = np.empty((N, N_NEIGHBORS), np.int64)
    chunk = 512
    for s in range(0, N, chunk):
        e = min(s + chunk, N)
        dx = qx[s:e, None] - cx[None, :]
        dy = qy[s:e, None] - cy[None, :]
        sc = np.full((e - s, N), -np.inf, np.float32)
        sc[:, valid_idx] = -(dx * dx + dy * dy)
        order = np.argsort(-sc, axis=1, kind="stable")
        pos4[s:e] = order[:, :N_NEIGHBORS]
    return pos4  # already pixel indices (full-N score rows)


def _pack_ab(Arows, Bblocks, C):
    """Assemble per-core AB matrices. Bblocks: [N_CORES][nb] of [K, C]."""
    import ml_dtypes

    Kdim = Arows.shape[0]
    ABmat = []
    for c in range(N_CORES):
        nb = len(Bblocks[c])
        ab = np.empty((Kdim, QPC + nb * C), np.float64)
        ab[:, :QPC] = Arows[:, c * QPC:(c + 1) * QPC]
        for i, blk in enumerate(Bblocks[c]):
            ab[:, QPC + i * C:QPC + (i + 1) * C] = blk
        ABmat.append(np.ascontiguousarray(ab.astype(ml_dtypes.bfloat16)))
    return ABmat


def kernel(S):
    S = np.asarray(S)
    flat = S.reshape(-1).astype(np.float32)
    valid_idx = np.flatnonzero(flat > V_THRESH)
    n_valid = int(valid_idx.size)

    if n_valid < 8 or n_valid > MAX_C:
        args_nq = _host_fallback(flat, valid_idx)
    else:
        cy = valid_idx // W
        R = _row_radius(valid_idx)                         # (H,)
        bands = [valid_idx[(cy >= y0 - R[y0]) & (cy <= y0 + R[y0])]
                 for y0 in range(H)]
        c_band = max(b.size for b in bands)
        C_tile = max(64, ((c_band + 63) // 64) * 64)

        if C_tile <= MAX_C_KEYED and C_tile < n_valid:
            # banded keyed path: per-tile candidate y-bands
            Arows = _query_features(True)
            Bblocks = [[_cand_features(True, C_tile, bands[12 * c + i])
                        for i in range(TILES)] for c in range(N_CORES)]
            out = _run_device(_pack_ab(Arows, Bblocks, C_tile),
                              C_tile, True, True)
            keys = out[:, :N_NEIGHBORS].astype(np.float64)
            pos = np.mod(-keys, KEY_M).astype(np.int64)    # (N, 4) local
            band_arr = np.zeros((H, C_tile), np.int64)
            for y0 in range(H):
                band_arr[y0, :bands[y0].size] = bands[y0]
            args_nq = band_arr[(np.arange(N) // W)[:, None], pos]
        else:
            # shared full candidate set
            C = max(P, ((n_valid + P - 1) // P) * P)
            keyed = C <= MAX_C_KEYED
            Arows = _query_features(keyed)
            Bblock = _cand_features(keyed, C, valid_idx)
            out = _run_device(_pack_ab(Arows, [[Bblock]] * N_CORES, C),
                              C, keyed, False)
            if keyed:
                keys = out[:, :N_NEIGHBORS].astype(np.float64)
                pos = np.mod(-keys, KEY_M).astype(np.int64)
            else:
                pos = out[:, :N_NEIGHBORS].astype(np.int64)
            args_nq = valid_idx[pos]

    args = args_nq.T.astype(np.int32)[None]                # (1, 4, N)
    ipc = np.empty((1, 2, N_NEIGHBORS, N), np.float32)
    ipc[0, 0] = (args[0] % W).astype(np.float32)
    ipc[0, 1] = (args[0] // W).astype(np.float32)
    return ipc, args


# revision 17
# speedup vs baseline: 2.2891x; 1.0098x over previous
"""Trainium2 Bass kernel for sparse-depth k-NN (nn_Dist).

For every pixel q of a 96x128 grid, find the 4 nearest valid pixels
(S > 0.001) by Euclidean distance, with jax.lax.top_k tie-breaking
(equal distance -> lowest linear index first).

Device algorithm (8 NeuronCores, SPMD over query rows, 1536 queries/core,
12 tiles of 128 queries = one pixel row per tile): the TensorEngine
computes, for each query q and candidate c,

    key(q, c) = 640 * (2*qx*cx + 2*qy*cy - cx^2 - cy^2) - idx_c
              = 640 * (-|q-c|^2 + qx^2 + qy^2) - idx_c

as a bf16 matmul (K=7: every factor split into bf16-exact integer parts)
accumulated in fp32 PSUM. Every product / partial sum stays an exact fp32
integer under either PE accumulation direction, so keys are EXACT. Keys
order candidates per query by (distance asc, index asc) — exactly
jax.lax.top_k order — and are unique, so the VectorEngine MAX8 instruction
alone (top-8 values per partition, read straight from PSUM) yields the
top-4; the host decodes idx = (-key) mod 640.

Candidate pruning: each tile is one pixel row y0. A cell-ring bound (8x8
cells; the smallest ring around a cell holding >= 4 valid pixels bounds
every in-cell query's 4th-NN distance) gives a per-row radius R(y0); any
candidate with |cy - y0| > R cannot be in that row's top-4, so each tile
only scores its y-band of candidates (typically ~halving the MAX8 stream).

For candidate counts > 640 the scaled key would overflow the 2^24
exact-integer range, so a fallback variant computes the unscaled score
(K=4 bf16 matmul) and uses MAX8 + MAX_INDEX (HW tie-break = first
occurrence = lowest index, verified exact vs top_k on HW).

Raw Bass with explicit semaphores: the Tile scheduler emits multiple
embedded sync-waits on Matmult instructions, which walrus codegen rejects
(the PE LDWEIGHTS struct holds one); standalone wait_ge ops avoid that.
"""

import numpy as np

H, W = 96, 128
N = H * W                    # 12288 queries
N_NEIGHBORS = 4
V_THRESH = 0.001
N_CORES = 8
QPC = N // N_CORES           # 1536 queries per core
P = 128                      # partitions
TILES = QPC // P             # 12 query tiles (pixel rows) per core
KEY_M = 640.0                # key multiplier; KEY_M*(25154+1) < 2^24
MAX_C_KEYED = 640            # idx < KEY_M and exactness both need C <= 640
MAX_C = 4096                 # PSUM free-dim capacity (fp32)
PSUM_WORDS = 4096            # fp32 words per partition in all 8 banks

_module_cache = {}
LAST_RESULTS = None  # BassKernelResults of the most recent device run


def _build_module(C, keyed, per_tile_b):
    """Raw-Bass module for C candidate columns.

    keyed=True : K=7 matmul of index-encoded keys, MAX8 only, fp32 out.
    keyed=False: K=4 matmul of plain scores, MAX8 + MAX_INDEX, uint32 out.
    per_tile_b : each tile has its own C candidate columns (y-band pruning).
    """
    import concourse.bass as bass
    import concourse.mybir as mybir

    f32 = mybir.dt.float32
    u32 = mybir.dt.uint32
    bf16 = mybir.dt.bfloat16
    K = 7 if keyed else 4
    nb = TILES if per_tile_b else 1

    slot_words = 512
    while slot_words < C:
        slot_words *= 2
    n_slots = max(1, PSUM_WORDS // slot_words)

    nc = bass.Bass(enable_partition_id=False, enable_asserts=False,
                   monotonic_sem_count=0)
    AB = nc.dram_tensor("AB", [K, QPC + nb * C], bf16, kind="ExternalInput")
    out_dt = f32 if keyed else u32
    OUT = nc.dram_tensor("OUT", [P, TILES * 8], out_dt, kind="ExternalOutput")

    with (
        nc.sbuf_tensor("ab_t", [K, QPC + nb * C], bf16) as ab_t,
        nc.sbuf_tensor("mx_all", [P, TILES * 8], f32) as mx_all,
        nc.psum_tensor("ps", [P, PSUM_WORDS], f32) as ps,
        nc.semaphore("dma_in") as dma_in,
        nc.semaphore("dma_in2") as dma_in2,
        nc.semaphore("pe_sem") as pe_sem,
        nc.semaphore("dve_sem") as dve_sem,
        nc.semaphore("dma_out") as dma_out,
    ):
        if keyed:
            _emit(nc, C, nb, slot_words, n_slots, AB, OUT, ab_t, mx_all,
                  None, ps, dma_in, dma_in2, pe_sem, dve_sem, dma_out,
                  keyed=True)
        else:
            with nc.sbuf_tensor("ix_all", [P, TILES * 8], u32) as ix_all:
                _emit(nc, C, nb, slot_words, n_slots, AB, OUT, ab_t, mx_all,
                      ix_all, ps, dma_in, dma_in2, pe_sem, dve_sem, dma_out,
                      keyed=False)
    return nc


def _emit(nc, C, nb, slot_words, n_slots, AB, OUT, ab_t, mx_all, ix_all,
          ps, dma_in, dma_in2, pe_sem, dve_sem, dma_out, keyed):
    dve_per_tile = 1 if keyed else 2
    out_sb = mx_all if keyed else ix_all
    # column layout: nb>1 -> [B0 | A | B1..B11] so one small DMA covers
    # everything tile 0 needs; nb==1 -> [A | B].
    a_off = C if nb > 1 else 0
    cut = C + QPC if nb > 1 else QPC + C          # first-DMA extent

    def boff(i):
        if nb == 1:
            return QPC
        return 0 if i == 0 else C + QPC + (i - 1) * C

    n_out = 4                                     # output DMA chunks
    step = TILES // n_out

    with nc.Block() as block:

        @block.sync
        def _(sync):
            sync.dma_start(ab_t[:, :cut], AB[:, :cut]).then_inc(dma_in, 16)
            if nb > 1:
                sync.dma_start(ab_t[:, cut:], AB[:, cut:]).then_inc(
                    dma_in2, 16)
            # ship results in chunks so the final DMA is small
            for k in range(n_out):
                sync.wait_ge(dve_sem, dve_per_tile * step * (k + 1))
                sl = slice(step * 8 * k, step * 8 * (k + 1))
                sync.dma_start(OUT[:, sl], out_sb[:, sl]).then_inc(
                    dma_out, 16)
            sync.wait_ge(dma_out, 16 * n_out)

        @block.tensor
        def _(tensor):
            # probe: timestamps when the PE body becomes executable
            tensor.wait_ge(dve_sem, 0)
            tensor.wait_ge(dma_in, 16)
            for i in range(TILES):
                if i == 1 and nb > 1:
                    tensor.wait_ge(dma_in2, 16)
                if i >= n_slots:
                    # PSUM slot (i % n_slots) must be drained by the DVE
                    tensor.wait_ge(
                        dve_sem, dve_per_tile * (i - n_slots + 1))
                base = (i % n_slots) * slot_words
                lhsT = ab_t[:, a_off + i * P:a_off + (i + 1) * P]
                last = None
                for j0 in range(0, C, 512):
                    j1 = min(j0 + 512, C)
                    last = tensor.matmul(
                        ps[:, base + j0:base + j1],
                        lhsT,
                        ab_t[:, boff(i) + j0:boff(i) + j1],
                    )
                last.then_inc(pe_sem)

        @block.vector
        def _(vector):
            for i in range(TILES):
                vector.wait_ge(pe_sem, i + 1)
                base = (i % n_slots) * slot_words
                sc_i = ps[:, base:base + C]
                vector.max(
                    out=mx_all[:, i * 8:(i + 1) * 8], in_=sc_i
                ).then_inc(dve_sem)
                if not keyed:
                    vector.wait_ge(dve_sem, 2 * i + 1)
                    vector.max_index(
                        out=ix_all[:, i * 8:(i + 1) * 8],
                        in_max=mx_all[:, i * 8:(i + 1) * 8],
                        in_values=sc_i,
                    ).then_inc(dve_sem)


def _get_module(C, keyed, per_tile_b):
    key = (C, keyed, per_tile_b)
    if key not in _module_cache:
        _module_cache[key] = _build_module(C, keyed, per_tile_b)
    return _module_cache[key]


def _run_device(ABmat, C, keyed, per_tile_b):
    """ABmat: list of [K, cols] bf16 per core -> (N, 8) out values."""
    from concourse.bass_utils import run_bass_kernel_spmd

    nc = _get_module(C, keyed, per_tile_b)
    in_maps = [{"AB": ab} for ab in ABmat]
    res = run_bass_kernel_spmd(nc, in_maps, core_ids=list(range(N_CORES)))
    global LAST_RESULTS
    LAST_RESULTS = res
    outs = []
    for r in res.results:
        o = r["OUT"].reshape(P, TILES, 8)          # [p, tile, rank]
        outs.append(o.transpose(1, 0, 2).reshape(QPC, 8))
    return np.concatenate(outs, axis=0)


def _query_features(keyed):
    """Per-query lhsT rows [K, N] as float (bf16-exact integer values)."""
    q = np.arange(N)
    qx = (q % W).astype(np.float64)
    qy = (q // W).astype(np.float64)
    if keyed:
        # 2*KEY_M*qx = 1280*qx split as 20480*(qx>>4) + 1280*(qx&15).
        # K order chosen so partial sums stay exact fp32 integers under
        # either PE accumulation direction.
        rows = [
            20480.0 * np.floor(qx / 16),   # * cx
            np.full(N, -65536.0),          # * v2
            20480.0 * np.floor(qy / 16),   # * cy
            1280.0 * (qx % 16),            # * cx
            1280.0 * (qy % 16),            # * cy
            np.full(N, -256.0),            # * v1
            np.full(N, -1.0),              # * v0
        ]
    else:
        rows = [
            2.0 * qx,                      # * cx
            2.0 * qy,                      # * cy
            np.full(N, -256.0),            # * w1
            np.full(N, -1.0),              # * w0
        ]
    return np.stack(rows)                  # [K, N]


def _cand_features(keyed, C, cand_idx):
    """Per-candidate rhs rows [K, C] incl. padding columns.

    cand_idx: pixel indices of this block's candidates (ascending)."""
    n = cand_idx.size
    cx = (cand_idx % W).astype(np.float64)
    cy = (cand_idx // W).astype(np.float64)
    Bm = np.zeros((7 if keyed else 4, C), np.float64)
    if keyed:
        v = KEY_M * (cx * cx + cy * cy) + np.arange(n, dtype=np.float64)
        assert n <= KEY_M and v.max(initial=0) < 2 ** 24
        Bm[0, :n] = cx
        Bm[1, :n] = np.floor(v / 65536)
        Bm[2, :n] = cy
        Bm[3, :n] = cx
        Bm[4, :n] = cy
        Bm[5, :n] = np.floor(v / 256) % 256
        Bm[6, :n] = v % 256
        # padding: key = -(65536+256+1)*255 = -16777215 < any real key
        Bm[1, n:] = 255.0
        Bm[5, n:] = 255.0
        Bm[6, n:] = 255.0
    else:
        w = cx * cx + cy * cy              # <= 25154
        Bm[0, :n] = cx
        Bm[1, :n] = cy
        Bm[2, :n] = np.floor(w / 256)
        Bm[3, :n] = w % 256
        # padding: score = -65535 < real score min (-25154)
        Bm[2, n:] = 255.0
        Bm[3, n:] = 255.0
    return Bm


def _row_radius_cells(valid_idx, cs):
    """Cell-ring bound with cell size cs: per pixel row y0, a radius R such
    that every query in row y0 has its 4 nearest valid pixels within
    |cy - y0| <= R (exact upper bound on the 4th-NN distance)."""
    cx = valid_idx % W
    cy = valid_idx // W
    gj, gi = -(-H // cs), -(-W // cs)
    cnt = np.zeros((gj, gi), np.int64)
    np.add.at(cnt, (cy // cs, cx // cs), 1)
    pre = np.zeros((gj + 1, gi + 1), np.int64)
    pre[1:, 1:] = cnt.cumsum(0).cumsum(1)

    def box(j0, j1, i0, i1):                      # inclusive cell box sum
        j0 = max(j0, 0); i0 = max(i0, 0)
        j1 = min(j1, gj - 1); i1 = min(i1, gi - 1)
        return pre[j1 + 1, i1 + 1] - pre[j0, i1 + 1] \
            - pre[j1 + 1, i0] + pre[j0, i0]

    bound = np.zeros((gj, gi))
    for j in range(gj):
        for i in range(gi):
            for r in range(max(gj, gi) + 1):
                if box(j - r, j + r, i - r, i + r) >= N_NEIGHBORS:
                    bound[j, i] = np.sqrt(2.0) * (cs * (r + 1) - 1)
                    break
            else:
                bound[j, i] = np.sqrt(2.0) * cs * (max(gj, gi) + 1)
    row_bound = bound.max(axis=1)                 # per cell row
    return np.ceil(row_bound[np.arange(H) // cs]).astype(np.int64)


def _row_radius(valid_idx):
    return np.minimum(_row_radius_cells(valid_idx, 8),
                      _row_radius_cells(valid_idx, 4))


def _host_fallback(flat, valid_idx):
    """Exact numpy replication of the reference for degenerate inputs."""
    q = np.arange(N)
    qx = (q % W).astype(np.float32)
    qy = (q // W).astype(np.float32)
    cx = (valid_idx % W).astype(np.float32)
    cy = (valid_idx // W).astype(np.float32)
    pos4 = np.empty((N, N_NEIGHBORS), np.int64)
    chunk = 512
    for s in range(0, N, chunk):
        e = min(s + chunk, N)
        dx = qx[s:e, None] - cx[None, :]
        dy = qy[s:e, None] - cy[None, :]
        sc = np.full((e - s, N), -np.inf, np.float32)
        sc[:, valid_idx] = -(dx * dx + dy * dy)
        order = np.argsort(-sc, axis=1, kind="stable")
        pos4[s:e] = order[:, :N_NEIGHBORS]
    return pos4  # already pixel indices (full-N score rows)


def _pack_ab(Arows, Bblocks, C):
    """Assemble per-core AB matrices. Bblocks: [N_CORES][nb] of [K, C].
    nb>1 layout: [B0 | A | B1..B11]; nb==1 layout: [A | B]."""
    import ml_dtypes

    Kdim = Arows.shape[0]
    ABmat = []
    for c in range(N_CORES):
        nb = len(Bblocks[c])
        ab = np.empty((Kdim, QPC + nb * C), np.float64)
        if nb == 1:
            ab[:, :QPC] = Arows[:, c * QPC:(c + 1) * QPC]
            ab[:, QPC:] = Bblocks[c][0]
        else:
            ab[:, :C] = Bblocks[c][0]
            ab[:, C:C + QPC] = Arows[:, c * QPC:(c + 1) * QPC]
            for i in range(1, nb):
                ab[:, C + QPC + (i - 1) * C:C + QPC + i * C] = Bblocks[c][i]
        ABmat.append(np.ascontiguousarray(ab.astype(ml_dtypes.bfloat16)))
    return ABmat


def kernel(S):
    S = np.asarray(S)
    flat = S.reshape(-1).astype(np.float32)
    valid_idx = np.flatnonzero(flat > V_THRESH)
    n_valid = int(valid_idx.size)

    if n_valid < 8 or n_valid > MAX_C:
        args_nq = _host_fallback(flat, valid_idx)
    else:
        cy = valid_idx // W
        R = _row_radius(valid_idx)                         # (H,)
        bands = [valid_idx[(cy >= y0 - R[y0]) & (cy <= y0 + R[y0])]
                 for y0 in range(H)]
        c_band = max(b.size for b in bands)
        C_tile = max(64, ((c_band + 63) // 64) * 64)

        if C_tile <= MAX_C_KEYED and C_tile < n_valid:
            # banded keyed path: per-tile candidate y-bands
            Arows = _query_features(True)
            Bblocks = [[_cand_features(True, C_tile, bands[12 * c + i])
                        for i in range(TILES)] for c in range(N_CORES)]
            out = _run_device(_pack_ab(Arows, Bblocks, C_tile),
                              C_tile, True, True)
            keys = out[:, :N_NEIGHBORS].astype(np.float64)
            pos = np.mod(-keys, KEY_M).astype(np.int64)    # (N, 4) local
            band_arr = np.zeros((H, C_tile), np.int64)
            for y0 in range(H):
                band_arr[y0, :bands[y0].size] = bands[y0]
            args_nq = band_arr[(np.arange(N) // W)[:, None], pos]
        else:
            # shared full candidate set
            C = max(P, ((n_valid + P - 1) // P) * P)
            keyed = C <= MAX_C_KEYED
            Arows = _query_features(keyed)
            Bblock = _cand_features(keyed, C, valid_idx)
            out = _run_device(_pack_ab(Arows, [[Bblock]] * N_CORES, C),
                              C, keyed, False)
            if keyed:
                keys = out[:, :N_NEIGHBORS].astype(np.float64)
                pos = np.mod(-keys, KEY_M).astype(np.int64)
            else:
                pos = out[:, :N_NEIGHBORS].astype(np.int64)
            args_nq = valid_idx[pos]

    args = args_nq.T.astype(np.int32)[None]                # (1, 4, N)
    ipc = np.empty((1, 2, N_NEIGHBORS, N), np.float32)
    ipc[0, 0] = (args[0] % W).astype(np.float32)
    ipc[0, 1] = (args[0] // W).astype(np.float32)
    return ipc, args


# revision 19
# speedup vs baseline: 2.3478x; 1.0256x over previous
"""Trainium2 Bass kernel for sparse-depth k-NN (nn_Dist).

For every pixel q of a 96x128 grid, find the 4 nearest valid pixels
(S > 0.001) by Euclidean distance, with jax.lax.top_k tie-breaking
(equal distance -> lowest linear index first).

Device algorithm (8 NeuronCores, SPMD over query rows, 1536 queries/core,
12 tiles of 128 queries = one pixel row per tile): the TensorEngine
computes, for each query q and candidate c,

    key(q, c) = 640 * (2*qx*cx + 2*qy*cy - cx^2 - cy^2) - idx_c
              = 640 * (-|q-c|^2 + qx^2 + qy^2) - idx_c

as a bf16 matmul (K=7: every factor split into bf16-exact integer parts)
accumulated in fp32 PSUM. Every product / partial sum stays an exact fp32
integer under either PE accumulation direction, so keys are EXACT. Keys
order candidates per query by (distance asc, index asc) — exactly
jax.lax.top_k order — and are unique, so the VectorEngine MAX8 instruction
alone (top-8 values per partition, read straight from PSUM) yields the
top-4; the host decodes idx = (-key) mod 640.

Candidate pruning: each tile is one pixel row y0. A cell-ring bound (8x8
cells; the smallest ring around a cell holding >= 4 valid pixels bounds
every in-cell query's 4th-NN distance) gives a per-row radius R(y0); any
candidate with |cy - y0| > R cannot be in that row's top-4, so each tile
only scores its y-band of candidates (typically ~halving the MAX8 stream).

For candidate counts > 640 the scaled key would overflow the 2^24
exact-integer range, so a fallback variant computes the unscaled score
(K=4 bf16 matmul) and uses MAX8 + MAX_INDEX (HW tie-break = first
occurrence = lowest index, verified exact vs top_k on HW).

Raw Bass with explicit semaphores: the Tile scheduler emits multiple
embedded sync-waits on Matmult instructions, which walrus codegen rejects
(the PE LDWEIGHTS struct holds one); standalone wait_ge ops avoid that.
"""

import numpy as np

H, W = 96, 128
N = H * W                    # 12288 queries
N_NEIGHBORS = 4
V_THRESH = 0.001
N_CORES = 8
QPC = N // N_CORES           # 1536 queries per core
P = 128                      # partitions
TILES = QPC // P             # 12 query tiles (pixel rows) per core
KEY_M = 640.0                # key multiplier; KEY_M*(25154+1) < 2^24
MAX_C_KEYED = 640            # idx < KEY_M and exactness both need C <= 640
MAX_C = 4096                 # PSUM free-dim capacity (fp32)
PSUM_WORDS = 4096            # fp32 words per partition in all 8 banks

_module_cache = {}
LAST_RESULTS = None  # BassKernelResults of the most recent device run


def _build_module(C, keyed, per_tile_b):
    """Raw-Bass module for C candidate columns.

    keyed=True : K=7 matmul of index-encoded keys, MAX8 only, fp32 out.
    keyed=False: K=4 matmul of plain scores, MAX8 + MAX_INDEX, uint32 out.
    per_tile_b : each tile has its own C candidate columns (y-band pruning).
    """
    import concourse.bass as bass
    import concourse.mybir as mybir

    f32 = mybir.dt.float32
    u32 = mybir.dt.uint32
    bf16 = mybir.dt.bfloat16
    K = 7 if keyed else 4
    nb = TILES if per_tile_b else 1

    slot_words = 512
    while slot_words < C:
        slot_words *= 2
    n_slots = max(1, PSUM_WORDS // slot_words)

    nc = bass.Bass(enable_partition_id=False, enable_asserts=False,
                   monotonic_sem_count=0)
    AB = nc.dram_tensor("AB", [K, QPC + nb * C], bf16, kind="ExternalInput")
    out_dt = f32 if keyed else u32
    OUT = nc.dram_tensor("OUT", [P, TILES * 8], out_dt, kind="ExternalOutput")

    with (
        nc.sbuf_tensor("ab_t", [K, QPC + nb * C], bf16) as ab_t,
        nc.sbuf_tensor("mx_all", [P, TILES * 8], f32) as mx_all,
        nc.psum_tensor("ps", [P, PSUM_WORDS], f32) as ps,
        nc.semaphore("dma_in") as dma_in,
        nc.semaphore("dma_in2") as dma_in2,
        nc.semaphore("pe_sem") as pe_sem,
        nc.semaphore("dve_sem") as dve_sem,
        nc.semaphore("dma_out") as dma_out,
    ):
        if keyed:
            _emit(nc, C, nb, slot_words, n_slots, AB, OUT, ab_t, mx_all,
                  None, ps, dma_in, dma_in2, pe_sem, dve_sem, dma_out,
                  keyed=True)
        else:
            with nc.sbuf_tensor("ix_all", [P, TILES * 8], u32) as ix_all:
                _emit(nc, C, nb, slot_words, n_slots, AB, OUT, ab_t, mx_all,
                      ix_all, ps, dma_in, dma_in2, pe_sem, dve_sem, dma_out,
                      keyed=False)
    return nc


def _emit(nc, C, nb, slot_words, n_slots, AB, OUT, ab_t, mx_all, ix_all,
          ps, dma_in, dma_in2, pe_sem, dve_sem, dma_out, keyed):
    dve_per_tile = 1 if keyed else 2
    out_sb = mx_all if keyed else ix_all
    # column layout: nb>1 -> [B0 | A | B1..B11] so one small DMA covers
    # everything tile 0 needs; nb==1 -> [A | B].
    a_off = C if nb > 1 else 0
    cut = C + QPC if nb > 1 else QPC + C          # first-DMA extent

    def boff(i):
        if nb == 1:
            return QPC
        return 0 if i == 0 else C + QPC + (i - 1) * C

    n_out = 4                                     # output DMA chunks
    step = TILES // n_out

    with nc.Block() as block:

        @block.gpsimd
        def _(gpsimd):
            # SWDGE queue: starts streaming sooner than the SP HWDGE path
            gpsimd.dma_start(ab_t[:, :cut], AB[:, :cut]).then_inc(dma_in, 16)

        if nb > 1:

            @block.scalar
            def _(scalar):
                # parallel HWDGE queue for the bulk of the candidate blocks
                scalar.dma_start(ab_t[:, cut:], AB[:, cut:]).then_inc(
                    dma_in2, 16)

        @block.sync
        def _(sync):
            # ship results in chunks so the final DMA is small
            for k in range(n_out):
                sync.wait_ge(dve_sem, dve_per_tile * step * (k + 1))
                sl = slice(step * 8 * k, step * 8 * (k + 1))
                sync.dma_start(OUT[:, sl], out_sb[:, sl]).then_inc(
                    dma_out, 16)
            sync.wait_ge(dma_out, 16 * n_out)

        @block.tensor
        def _(tensor):
            # probe: timestamps when the PE body becomes executable
            tensor.wait_ge(dve_sem, 0)
            tensor.wait_ge(dma_in, 16)
            for i in range(TILES):
                if i == 1 and nb > 1:
                    tensor.wait_ge(dma_in2, 16)
                if i >= n_slots:
                    # PSUM slot (i % n_slots) must be drained by the DVE
                    tensor.wait_ge(
                        dve_sem, dve_per_tile * (i - n_slots + 1))
                base = (i % n_slots) * slot_words
                lhsT = ab_t[:, a_off + i * P:a_off + (i + 1) * P]
                last = None
                for j0 in range(0, C, 512):
                    j1 = min(j0 + 512, C)
                    last = tensor.matmul(
                        ps[:, base + j0:base + j1],
                        lhsT,
                        ab_t[:, boff(i) + j0:boff(i) + j1],
                    )
                last.then_inc(pe_sem)

        @block.vector
        def _(vector):
            for i in range(TILES):
                vector.wait_ge(pe_sem, i + 1)
                base = (i % n_slots) * slot_words
                sc_i = ps[:, base:base + C]
                vector.max(
                    out=mx_all[:, i * 8:(i + 1) * 8], in_=sc_i
                ).then_inc(dve_sem)
                if not keyed:
                    vector.wait_ge(dve_sem, 2 * i + 1)
                    vector.max_index(
                        out=ix_all[:, i * 8:(i + 1) * 8],
                        in_max=mx_all[:, i * 8:(i + 1) * 8],
                        in_values=sc_i,
                    ).then_inc(dve_sem)


def _get_module(C, keyed, per_tile_b):
    key = (C, keyed, per_tile_b)
    if key not in _module_cache:
        _module_cache[key] = _build_module(C, keyed, per_tile_b)
    return _module_cache[key]


def _run_device(ABmat, C, keyed, per_tile_b):
    """ABmat: list of [K, cols] bf16 per core -> (N, 8) out values."""
    from concourse.bass_utils import run_bass_kernel_spmd

    nc = _get_module(C, keyed, per_tile_b)
    in_maps = [{"AB": ab} for ab in ABmat]
    res = run_bass_kernel_spmd(nc, in_maps, core_ids=list(range(N_CORES)))
    global LAST_RESULTS
    LAST_RESULTS = res
    outs = []
    for r in res.results:
        o = r["OUT"].reshape(P, TILES, 8)          # [p, tile, rank]
        outs.append(o.transpose(1, 0, 2).reshape(QPC, 8))
    return np.concatenate(outs, axis=0)


def _query_features(keyed):
    """Per-query lhsT rows [K, N] as float (bf16-exact integer values)."""
    q = np.arange(N)
    qx = (q % W).astype(np.float64)
    qy = (q // W).astype(np.float64)
    if keyed:
        # 2*KEY_M*qx = 1280*qx split as 20480*(qx>>4) + 1280*(qx&15).
        # K order chosen so partial sums stay exact fp32 integers under
        # either PE accumulation direction.
        rows = [
            20480.0 * np.floor(qx / 16),   # * cx
            np.full(N, -65536.0),          # * v2
            20480.0 * np.floor(qy / 16),   # * cy
            1280.0 * (qx % 16),            # * cx
            1280.0 * (qy % 16),            # * cy
            np.full(N, -256.0),            # * v1
            np.full(N, -1.0),              # * v0
        ]
    else:
        rows = [
            2.0 * qx,                      # * cx
            2.0 * qy,                      # * cy
            np.full(N, -256.0),            # * w1
            np.full(N, -1.0),              # * w0
        ]
    return np.stack(rows)                  # [K, N]


def _cand_features(keyed, C, cand_idx):
    """Per-candidate rhs rows [K, C] incl. padding columns.

    cand_idx: pixel indices of this block's candidates (ascending)."""
    n = cand_idx.size
    cx = (cand_idx % W).astype(np.float64)
    cy = (cand_idx // W).astype(np.float64)
    Bm = np.zeros((7 if keyed else 4, C), np.float64)
    if keyed:
        v = KEY_M * (cx * cx + cy * cy) + np.arange(n, dtype=np.float64)
        assert n <= KEY_M and v.max(initial=0) < 2 ** 24
        Bm[0, :n] = cx
        Bm[1, :n] = np.floor(v / 65536)
        Bm[2, :n] = cy
        Bm[3, :n] = cx
        Bm[4, :n] = cy
        Bm[5, :n] = np.floor(v / 256) % 256
        Bm[6, :n] = v % 256
        # padding: key = -(65536+256+1)*255 = -16777215 < any real key
        Bm[1, n:] = 255.0
        Bm[5, n:] = 255.0
        Bm[6, n:] = 255.0
    else:
        w = cx * cx + cy * cy              # <= 25154
        Bm[0, :n] = cx
        Bm[1, :n] = cy
        Bm[2, :n] = np.floor(w / 256)
        Bm[3, :n] = w % 256
        # padding: score = -65535 < real score min (-25154)
        Bm[2, n:] = 255.0
        Bm[3, n:] = 255.0
    return Bm


def _row_radius_cells(valid_idx, cs):
    """Cell-ring bound with cell size cs: per pixel row y0, a radius R such
    that every query in row y0 has its 4 nearest valid pixels within
    |cy - y0| <= R (exact upper bound on the 4th-NN distance)."""
    cx = valid_idx % W
    cy = valid_idx // W
    gj, gi = -(-H // cs), -(-W // cs)
    cnt = np.zeros((gj, gi), np.int64)
    np.add.at(cnt, (cy // cs, cx // cs), 1)
    pre = np.zeros((gj + 1, gi + 1), np.int64)
    pre[1:, 1:] = cnt.cumsum(0).cumsum(1)

    def box(j0, j1, i0, i1):                      # inclusive cell box sum
        j0 = max(j0, 0); i0 = max(i0, 0)
        j1 = min(j1, gj - 1); i1 = min(i1, gi - 1)
        return pre[j1 + 1, i1 + 1] - pre[j0, i1 + 1] \
            - pre[j1 + 1, i0] + pre[j0, i0]

    bound = np.zeros((gj, gi))
    for j in range(gj):
        for i in range(gi):
            for r in range(max(gj, gi) + 1):
                if box(j - r, j + r, i - r, i + r) >= N_NEIGHBORS:
                    bound[j, i] = np.sqrt(2.0) * (cs * (r + 1) - 1)
                    break
            else:
                bound[j, i] = np.sqrt(2.0) * cs * (max(gj, gi) + 1)
    row_bound = bound.max(axis=1)                 # per cell row
    return np.ceil(row_bound[np.arange(H) // cs]).astype(np.int64)


def _row_radius(valid_idx):
    r = np.minimum(_row_radius_cells(valid_idx, 8),
                   _row_radius_cells(valid_idx, 4))
    return np.minimum(r, _row_radius_cells(valid_idx, 2))


def _host_fallback(flat, valid_idx):
    """Exact numpy replication of the reference for degenerate inputs."""
    q = np.arange(N)
    qx = (q % W).astype(np.float32)
    qy = (q // W).astype(np.float32)
    cx = (valid_idx % W).astype(np.float32)
    cy = (valid_idx // W).astype(np.float32)
    pos4 = np.empty((N, N_NEIGHBORS), np.int64)
    chunk = 512
    for s in range(0, N, chunk):
        e = min(s + chunk, N)
        dx = qx[s:e, None] - cx[None, :]
        dy = qy[s:e, None] - cy[None, :]
        sc = np.full((e - s, N), -np.inf, np.float32)
        sc[:, valid_idx] = -(dx * dx + dy * dy)
        order = np.argsort(-sc, axis=1, kind="stable")
        pos4[s:e] = order[:, :N_NEIGHBORS]
    return pos4  # already pixel indices (full-N score rows)


def _pack_ab(Arows, Bblocks, C):
    """Assemble per-core AB matrices. Bblocks: [N_CORES][nb] of [K, C].
    nb>1 layout: [B0 | A | B1..B11]; nb==1 layout: [A | B]."""
    import ml_dtypes

    Kdim = Arows.shape[0]
    ABmat = []
    for c in range(N_CORES):
        nb = len(Bblocks[c])
        ab = np.empty((Kdim, QPC + nb * C), np.float64)
        if nb == 1:
            ab[:, :QPC] = Arows[:, c * QPC:(c + 1) * QPC]
            ab[:, QPC:] = Bblocks[c][0]
        else:
            ab[:, :C] = Bblocks[c][0]
            ab[:, C:C + QPC] = Arows[:, c * QPC:(c + 1) * QPC]
            for i in range(1, nb):
                ab[:, C + QPC + (i - 1) * C:C + QPC + i * C] = Bblocks[c][i]
        ABmat.append(np.ascontiguousarray(ab.astype(ml_dtypes.bfloat16)))
    return ABmat


def kernel(S):
    S = np.asarray(S)
    flat = S.reshape(-1).astype(np.float32)
    valid_idx = np.flatnonzero(flat > V_THRESH)
    n_valid = int(valid_idx.size)

    if n_valid < 8 or n_valid > MAX_C:
        args_nq = _host_fallback(flat, valid_idx)
    else:
        cy = valid_idx // W
        R = _row_radius(valid_idx)                         # (H,)
        bands = [valid_idx[(cy >= y0 - R[y0]) & (cy <= y0 + R[y0])]
                 for y0 in range(H)]
        c_band = max(b.size for b in bands)
        C_tile = max(64, ((c_band + 63) // 64) * 64)

        if C_tile <= MAX_C_KEYED and C_tile < n_valid:
            # banded keyed path: per-tile candidate y-bands
            Arows = _query_features(True)
            Bblocks = [[_cand_features(True, C_tile, bands[12 * c + i])
                        for i in range(TILES)] for c in range(N_CORES)]
            out = _run_device(_pack_ab(Arows, Bblocks, C_tile),
                              C_tile, True, True)
            keys = out[:, :N_NEIGHBORS].astype(np.float64)
            pos = np.mod(-keys, KEY_M).astype(np.int64)    # (N, 4) local
            band_arr = np.zeros((H, C_tile), np.int64)
            for y0 in range(H):
                band_arr[y0, :bands[y0].size] = bands[y0]
            args_nq = band_arr[(np.arange(N) // W)[:, None], pos]
        else:
            # shared full candidate set
            C = max(P, ((n_valid + P - 1) // P) * P)
            keyed = C <= MAX_C_KEYED
            Arows = _query_features(keyed)
            Bblock = _cand_features(keyed, C, valid_idx)
            out = _run_device(_pack_ab(Arows, [[Bblock]] * N_CORES, C),
                              C, keyed, False)
            if keyed:
                keys = out[:, :N_NEIGHBORS].astype(np.float64)
                pos = np.mod(-keys, KEY_M).astype(np.int64)
            else:
                pos = out[:, :N_NEIGHBORS].astype(np.int64)
            args_nq = valid_idx[pos]

    args = args_nq.T.astype(np.int32)[None]                # (1, 4, N)
    ipc = np.empty((1, 2, N_NEIGHBORS, N), np.float32)
    ipc[0, 0] = (args[0] % W).astype(np.float32)
    ipc[0, 1] = (args[0] // W).astype(np.float32)
    return ipc, args


# revision 20
# speedup vs baseline: 2.6281x; 1.1194x over previous
"""Trainium2 Bass kernel for sparse-depth k-NN (nn_Dist).

For every pixel q of a 96x128 grid, find the 4 nearest valid pixels
(S > 0.001) by Euclidean distance, with jax.lax.top_k tie-breaking
(equal distance -> lowest linear index first).

Device algorithm (8 NeuronCores, SPMD over query rows, 1536 queries/core,
12 tiles of 128 queries = one pixel row per tile): the TensorEngine
computes, for each query q and candidate c,

    key(q, c) = 640 * (2*qx*cx + 2*qy*cy - cx^2 - cy^2) - idx_c
              = 640 * (-|q-c|^2 + qx^2 + qy^2) - idx_c

as a bf16 matmul (K=7: every factor split into bf16-exact integer parts)
accumulated in fp32 PSUM. Every product / partial sum stays an exact fp32
integer under either PE accumulation direction, so keys are EXACT. Keys
order candidates per query by (distance asc, index asc) — exactly
jax.lax.top_k order — and are unique, so the VectorEngine MAX8 instruction
alone (top-8 values per partition, read straight from PSUM) yields the
top-4; the host decodes idx = (-key) mod 640.

Candidate pruning: each tile is one pixel row y0. A cell-ring bound (8x8
cells; the smallest ring around a cell holding >= 4 valid pixels bounds
every in-cell query's 4th-NN distance) gives a per-row radius R(y0); any
candidate with |cy - y0| > R cannot be in that row's top-4, so each tile
only scores its y-band of candidates (typically ~halving the MAX8 stream).

For candidate counts > 640 the scaled key would overflow the 2^24
exact-integer range, so a fallback variant computes the unscaled score
(K=4 bf16 matmul) and uses MAX8 + MAX_INDEX (HW tie-break = first
occurrence = lowest index, verified exact vs top_k on HW).

Raw Bass with explicit semaphores: the Tile scheduler emits multiple
embedded sync-waits on Matmult instructions, which walrus codegen rejects
(the PE LDWEIGHTS struct holds one); standalone wait_ge ops avoid that.
"""

import numpy as np

H, W = 96, 128
N = H * W                    # 12288 queries
N_NEIGHBORS = 4
V_THRESH = 0.001
N_CORES = 8
QPC = N // N_CORES           # 1536 queries per core
P = 128                      # partitions
TILES = QPC // P             # 12 query tiles (pixel rows) per core
KEY_M = 640.0                # key multiplier; KEY_M*(25154+1) < 2^24
MAX_C_KEYED = 640            # idx < KEY_M and exactness both need C <= 640
MAX_C = 4096                 # PSUM free-dim capacity (fp32)
PSUM_WORDS = 4096            # fp32 words per partition in all 8 banks

_module_cache = {}
LAST_RESULTS = None  # BassKernelResults of the most recent device run


def _build_module(C, keyed, per_tile_b):
    """Raw-Bass module for C candidate columns.

    keyed=True : K=7 matmul of index-encoded keys, MAX8 only, fp32 out.
    keyed=False: K=4 matmul of plain scores, MAX8 + MAX_INDEX, uint32 out.
    per_tile_b : each tile has its own C candidate columns (y-band pruning).
    """
    import concourse.bass as bass
    import concourse.mybir as mybir

    f32 = mybir.dt.float32
    u32 = mybir.dt.uint32
    bf16 = mybir.dt.bfloat16
    K = 7 if keyed else 4
    nb = TILES if per_tile_b else 1

    slot_words = 512
    while slot_words < C:
        slot_words *= 2
    n_slots = max(1, PSUM_WORDS // slot_words)

    nc = bass.Bass(enable_partition_id=False, enable_asserts=False,
                   monotonic_sem_count=0)
    AB = nc.dram_tensor("AB", [K, QPC + nb * C], bf16, kind="ExternalInput")
    out_dt = f32 if keyed else u32
    OUT = nc.dram_tensor("OUT", [P, TILES * 8], out_dt, kind="ExternalOutput")

    with (
        nc.sbuf_tensor("ab_t", [K, QPC + nb * C], bf16) as ab_t,
        nc.sbuf_tensor("mx_all", [P, TILES * 8], f32) as mx_all,
        nc.psum_tensor("ps", [P, PSUM_WORDS], f32) as ps,
        nc.semaphore("dma_in") as dma_in,
        nc.semaphore("dma_in2") as dma_in2,
        nc.semaphore("pe_sem") as pe_sem,
        nc.semaphore("dve_sem") as dve_sem,
        nc.semaphore("dma_out") as dma_out,
    ):
        if keyed:
            _emit(nc, C, nb, slot_words, n_slots, AB, OUT, ab_t, mx_all,
                  None, ps, dma_in, dma_in2, pe_sem, dve_sem, dma_out,
                  keyed=True)
        else:
            with nc.sbuf_tensor("ix_all", [P, TILES * 8], u32) as ix_all:
                _emit(nc, C, nb, slot_words, n_slots, AB, OUT, ab_t, mx_all,
                      ix_all, ps, dma_in, dma_in2, pe_sem, dve_sem, dma_out,
                      keyed=False)
    return nc


def _emit(nc, C, nb, slot_words, n_slots, AB, OUT, ab_t, mx_all, ix_all,
          ps, dma_in, dma_in2, pe_sem, dve_sem, dma_out, keyed):
    dve_per_tile = 1 if keyed else 2
    out_sb = mx_all if keyed else ix_all
    # column layout: nb>1 -> [B0 | A | B1..B11] so one small DMA covers
    # everything tile 0 needs; nb==1 -> [A | B].
    a_off = C if nb > 1 else 0
    cut = C + QPC if nb > 1 else QPC + C          # first-DMA extent

    def boff(i):
        if nb == 1:
            return QPC
        return 0 if i == 0 else C + QPC + (i - 1) * C

    n_out = 4                                     # output DMA chunks
    step = TILES // n_out

    with nc.Block() as block:

        if nb > 1:

            @block.scalar
            def _(scalar):
                # parallel HWDGE queue for the bulk of the candidate blocks
                scalar.dma_start(ab_t[:, cut:], AB[:, cut:]).then_inc(
                    dma_in2, 16)

        @block.sync
        def _(sync):
            sync.dma_start(ab_t[:, :cut], AB[:, :cut]).then_inc(dma_in, 16)
            # ship results in chunks so the final DMA is small; no final
            # wait on dma_out - the module epilogue drains the DMA queues,
            # overlapping the last chunk's completion latency with the
            # semaphore-reset storm.
            for k in range(n_out):
                sync.wait_ge(dve_sem, dve_per_tile * step * (k + 1))
                sl = slice(step * 8 * k, step * 8 * (k + 1))
                sync.dma_start(OUT[:, sl], out_sb[:, sl]).then_inc(
                    dma_out, 16)

        @block.tensor
        def _(tensor):
            # probe: timestamps when the PE body becomes executable
            tensor.wait_ge(dve_sem, 0)
            tensor.wait_ge(dma_in, 16)
            for i in range(TILES):
                if i == 1 and nb > 1:
                    tensor.wait_ge(dma_in2, 16)
                if i >= n_slots:
                    # PSUM slot (i % n_slots) must be drained by the DVE
                    tensor.wait_ge(
                        dve_sem, dve_per_tile * (i - n_slots + 1))
                base = (i % n_slots) * slot_words
                lhsT = ab_t[:, a_off + i * P:a_off + (i + 1) * P]
                last = None
                for j0 in range(0, C, 512):
                    j1 = min(j0 + 512, C)
                    last = tensor.matmul(
                        ps[:, base + j0:base + j1],
                        lhsT,
                        ab_t[:, boff(i) + j0:boff(i) + j1],
                    )
                last.then_inc(pe_sem)

        @block.vector
        def _(vector):
            for i in range(TILES):
                vector.wait_ge(pe_sem, i + 1)
                base = (i % n_slots) * slot_words
                sc_i = ps[:, base:base + C]
                vector.max(
                    out=mx_all[:, i * 8:(i + 1) * 8], in_=sc_i
                ).then_inc(dve_sem)
                if not keyed:
                    vector.wait_ge(dve_sem, 2 * i + 1)
                    vector.max_index(
                        out=ix_all[:, i * 8:(i + 1) * 8],
                        in_max=mx_all[:, i * 8:(i + 1) * 8],
                        in_values=sc_i,
                    ).then_inc(dve_sem)


def _get_module(C, keyed, per_tile_b):
    key = (C, keyed, per_tile_b)
    if key not in _module_cache:
        _module_cache[key] = _build_module(C, keyed, per_tile_b)
    return _module_cache[key]


def _run_device(ABmat, C, keyed, per_tile_b):
    """ABmat: list of [K, cols] bf16 per core -> (N, 8) out values."""
    from concourse.bass_utils import run_bass_kernel_spmd

    nc = _get_module(C, keyed, per_tile_b)
    in_maps = [{"AB": ab} for ab in ABmat]
    res = run_bass_kernel_spmd(nc, in_maps, core_ids=list(range(N_CORES)))
    global LAST_RESULTS
    LAST_RESULTS = res
    outs = []
    for r in res.results:
        o = r["OUT"].reshape(P, TILES, 8)          # [p, tile, rank]
        outs.append(o.transpose(1, 0, 2).reshape(QPC, 8))
    return np.concatenate(outs, axis=0)


def _query_features(keyed):
    """Per-query lhsT rows [K, N] as float (bf16-exact integer values)."""
    q = np.arange(N)
    qx = (q % W).astype(np.float64)
    qy = (q // W).astype(np.float64)
    if keyed:
        # 2*KEY_M*qx = 1280*qx split as 20480*(qx>>4) + 1280*(qx&15).
        # K order chosen so partial sums stay exact fp32 integers under
        # either PE accumulation direction.
        rows = [
            20480.0 * np.floor(qx / 16),   # * cx
            np.full(N, -65536.0),          # * v2
            20480.0 * np.floor(qy / 16),   # * cy
            1280.0 * (qx % 16),            # * cx
            1280.0 * (qy % 16),            # * cy
            np.full(N, -256.0),            # * v1
            np.full(N, -1.0),              # * v0
        ]
    else:
        rows = [
            2.0 * qx,                      # * cx
            2.0 * qy,                      # * cy
            np.full(N, -256.0),            # * w1
            np.full(N, -1.0),              # * w0
        ]
    return np.stack(rows)                  # [K, N]


def _cand_features(keyed, C, cand_idx):
    """Per-candidate rhs rows [K, C] incl. padding columns.

    cand_idx: pixel indices of this block's candidates (ascending)."""
    n = cand_idx.size
    cx = (cand_idx % W).astype(np.float64)
    cy = (cand_idx // W).astype(np.float64)
    Bm = np.zeros((7 if keyed else 4, C), np.float64)
    if keyed:
        v = KEY_M * (cx * cx + cy * cy) + np.arange(n, dtype=np.float64)
        assert n <= KEY_M and v.max(initial=0) < 2 ** 24
        Bm[0, :n] = cx
        Bm[1, :n] = np.floor(v / 65536)
        Bm[2, :n] = cy
        Bm[3, :n] = cx
        Bm[4, :n] = cy
        Bm[5, :n] = np.floor(v / 256) % 256
        Bm[6, :n] = v % 256
        # padding: key = -(65536+256+1)*255 = -16777215 < any real key
        Bm[1, n:] = 255.0
        Bm[5, n:] = 255.0
        Bm[6, n:] = 255.0
    else:
        w = cx * cx + cy * cy              # <= 25154
        Bm[0, :n] = cx
        Bm[1, :n] = cy
        Bm[2, :n] = np.floor(w / 256)
        Bm[3, :n] = w % 256
        # padding: score = -65535 < real score min (-25154)
        Bm[2, n:] = 255.0
        Bm[3, n:] = 255.0
    return Bm


def _row_radius_cells(valid_idx, cs):
    """Cell-ring bound with cell size cs: per pixel row y0, a radius R such
    that every query in row y0 has its 4 nearest valid pixels within
    |cy - y0| <= R (exact upper bound on the 4th-NN distance)."""
    cx = valid_idx % W
    cy = valid_idx // W
    gj, gi = -(-H // cs), -(-W // cs)
    cnt = np.zeros((gj, gi), np.int64)
    np.add.at(cnt, (cy // cs, cx // cs), 1)
    pre = np.zeros((gj + 1, gi + 1), np.int64)
    pre[1:, 1:] = cnt.cumsum(0).cumsum(1)

    def box(j0, j1, i0, i1):                      # inclusive cell box sum
        j0 = max(j0, 0); i0 = max(i0, 0)
        j1 = min(j1, gj - 1); i1 = min(i1, gi - 1)
        return pre[j1 + 1, i1 + 1] - pre[j0, i1 + 1] \
            - pre[j1 + 1, i0] + pre[j0, i0]

    bound = np.zeros((gj, gi))
    for j in range(gj):
        for i in range(gi):
            for r in range(max(gj, gi) + 1):
                if box(j - r, j + r, i - r, i + r) >= N_NEIGHBORS:
                    bound[j, i] = np.sqrt(2.0) * (cs * (r + 1) - 1)
                    break
            else:
                bound[j, i] = np.sqrt(2.0) * cs * (max(gj, gi) + 1)
    row_bound = bound.max(axis=1)                 # per cell row
    return np.ceil(row_bound[np.arange(H) // cs]).astype(np.int64)


def _row_radius(valid_idx):
    r = np.minimum(_row_radius_cells(valid_idx, 8),
                   _row_radius_cells(valid_idx, 4))
    return np.minimum(r, _row_radius_cells(valid_idx, 2))


def _host_fallback(flat, valid_idx):
    """Exact numpy replication of the reference for degenerate inputs."""
    q = np.arange(N)
    qx = (q % W).astype(np.float32)
    qy = (q // W).astype(np.float32)
    cx = (valid_idx % W).astype(np.float32)
    cy = (valid_idx // W).astype(np.float32)
    pos4 = np.empty((N, N_NEIGHBORS), np.int64)
    chunk = 512
    for s in range(0, N, chunk):
        e = min(s + chunk, N)
        dx = qx[s:e, None] - cx[None, :]
        dy = qy[s:e, None] - cy[None, :]
        sc = np.full((e - s, N), -np.inf, np.float32)
        sc[:, valid_idx] = -(dx * dx + dy * dy)
        order = np.argsort(-sc, axis=1, kind="stable")
        pos4[s:e] = order[:, :N_NEIGHBORS]
    return pos4  # already pixel indices (full-N score rows)


def _pack_ab(Arows, Bblocks, C):
    """Assemble per-core AB matrices. Bblocks: [N_CORES][nb] of [K, C].
    nb>1 layout: [B0 | A | B1..B11]; nb==1 layout: [A | B]."""
    import ml_dtypes

    Kdim = Arows.shape[0]
    ABmat = []
    for c in range(N_CORES):
        nb = len(Bblocks[c])
        ab = np.empty((Kdim, QPC + nb * C), np.float64)
        if nb == 1:
            ab[:, :QPC] = Arows[:, c * QPC:(c + 1) * QPC]
            ab[:, QPC:] = Bblocks[c][0]
        else:
            ab[:, :C] = Bblocks[c][0]
            ab[:, C:C + QPC] = Arows[:, c * QPC:(c + 1) * QPC]
            for i in range(1, nb):
                ab[:, C + QPC + (i - 1) * C:C + QPC + i * C] = Bblocks[c][i]
        ABmat.append(np.ascontiguousarray(ab.astype(ml_dtypes.bfloat16)))
    return ABmat


def kernel(S):
    S = np.asarray(S)
    flat = S.reshape(-1).astype(np.float32)
    valid_idx = np.flatnonzero(flat > V_THRESH)
    n_valid = int(valid_idx.size)

    if n_valid < 8 or n_valid > MAX_C:
        args_nq = _host_fallback(flat, valid_idx)
    else:
        cy = valid_idx // W
        R = _row_radius(valid_idx)                         # (H,)
        bands = [valid_idx[(cy >= y0 - R[y0]) & (cy <= y0 + R[y0])]
                 for y0 in range(H)]
        c_band = max(b.size for b in bands)
        C_tile = max(64, ((c_band + 63) // 64) * 64)

        if C_tile <= MAX_C_KEYED and C_tile < n_valid:
            # banded keyed path: per-tile candidate y-bands
            Arows = _query_features(True)
            Bblocks = [[_cand_features(True, C_tile, bands[12 * c + i])
                        for i in range(TILES)] for c in range(N_CORES)]
            out = _run_device(_pack_ab(Arows, Bblocks, C_tile),
                              C_tile, True, True)
            keys = out[:, :N_NEIGHBORS].astype(np.float64)
            pos = np.mod(-keys, KEY_M).astype(np.int64)    # (N, 4) local
            band_arr = np.zeros((H, C_tile), np.int64)
            for y0 in range(H):
                band_arr[y0, :bands[y0].size] = bands[y0]
            args_nq = band_arr[(np.arange(N) // W)[:, None], pos]
        else:
            # shared full candidate set
            C = max(P, ((n_valid + P - 1) // P) * P)
            keyed = C <= MAX_C_KEYED
            Arows = _query_features(keyed)
            Bblock = _cand_features(keyed, C, valid_idx)
            out = _run_device(_pack_ab(Arows, [[Bblock]] * N_CORES, C),
                              C, keyed, False)
            if keyed:
                keys = out[:, :N_NEIGHBORS].astype(np.float64)
                pos = np.mod(-keys, KEY_M).astype(np.int64)
            else:
                pos = out[:, :N_NEIGHBORS].astype(np.int64)
            args_nq = valid_idx[pos]

    args = args_nq.T.astype(np.int32)[None]                # (1, 4, N)
    ipc = np.empty((1, 2, N_NEIGHBORS, N), np.float32)
    ipc[0, 0] = (args[0] % W).astype(np.float32)
    ipc[0, 1] = (args[0] // W).astype(np.float32)
    return ipc, args


# revision 25
# speedup vs baseline: 2.7746x; 1.0558x over previous
"""Trainium2 Bass kernel for sparse-depth k-NN (nn_Dist).

For every pixel q of a 96x128 grid, find the 4 nearest valid pixels
(S > 0.001) by Euclidean distance, with jax.lax.top_k tie-breaking
(equal distance -> lowest linear index first).

Device algorithm (8 NeuronCores, SPMD over query rows, 1536 queries/core,
12 tiles of 128 queries = one pixel row per tile): the TensorEngine
computes, for each query q and candidate c,

    key(q, c) = 640 * (2*qx*cx + 2*qy*cy - cx^2 - cy^2) - idx_c
              = 640 * (-|q-c|^2 + qx^2 + qy^2) - idx_c

as a bf16 matmul (K=7: every factor split into bf16-exact integer parts)
accumulated in fp32 PSUM. Every product / partial sum stays an exact fp32
integer under either PE accumulation direction, so keys are EXACT. Keys
order candidates per query by (distance asc, index asc) — exactly
jax.lax.top_k order — and are unique, so the VectorEngine MAX8 instruction
alone (top-8 values per partition, read straight from PSUM) yields the
top-4; the host decodes idx = (-key) mod 640.

Candidate pruning: each tile is one pixel row y0. A cell-ring bound (8x8
cells; the smallest ring around a cell holding >= 4 valid pixels bounds
every in-cell query's 4th-NN distance) gives a per-row radius R(y0); any
candidate with |cy - y0| > R cannot be in that row's top-4, so each tile
only scores its y-band of candidates (typically ~halving the MAX8 stream).

For candidate counts > 640 the scaled key would overflow the 2^24
exact-integer range, so a fallback variant computes the unscaled score
(K=4 bf16 matmul) and uses MAX8 + MAX_INDEX (HW tie-break = first
occurrence = lowest index, verified exact vs top_k on HW).

Raw Bass with explicit semaphores: the Tile scheduler emits multiple
embedded sync-waits on Matmult instructions, which walrus codegen rejects
(the PE LDWEIGHTS struct holds one); standalone wait_ge ops avoid that.
"""

import numpy as np

H, W = 96, 128
N = H * W                    # 12288 queries
N_NEIGHBORS = 4
V_THRESH = 0.001
N_CORES = 8
QPC = N // N_CORES           # 1536 queries per core
P = 128                      # partitions
TILES = QPC // P             # 12 query tiles (pixel rows) per core
KEY_M = 640.0                # key multiplier; KEY_M*(25154+1) < 2^24
MAX_C_KEYED = 640            # idx < KEY_M and exactness both need C <= 640
MAX_C = 4096                 # PSUM free-dim capacity (fp32)
PSUM_WORDS = 4096            # fp32 words per partition in all 8 banks

_module_cache = {}
LAST_RESULTS = None  # BassKernelResults of the most recent device run


def _build_module(C, keyed, per_tile_b):
    """Raw-Bass module for C candidate columns.

    keyed=True : K=7 matmul of index-encoded keys, MAX8 only, fp32 out.
    keyed=False: K=4 matmul of plain scores, MAX8 + MAX_INDEX, uint32 out.
    per_tile_b : each tile has its own C candidate columns (y-band pruning).
    """
    import concourse.bass as bass
    import concourse.mybir as mybir

    f32 = mybir.dt.float32
    u32 = mybir.dt.uint32
    bf16 = mybir.dt.bfloat16
    K = 7 if keyed else 4
    nb = TILES if per_tile_b else 1

    slot_words = 512
    while slot_words < C:
        slot_words *= 2
    n_slots = max(1, PSUM_WORDS // slot_words)

    nc = bass.Bass(enable_partition_id=False, enable_asserts=False,
                   monotonic_sem_count=0)
    AB = nc.dram_tensor("AB", [K, QPC + nb * C], bf16, kind="ExternalInput")
    out_dt = f32 if keyed else u32
    OUT = nc.dram_tensor("OUT", [P, TILES * 8], out_dt, kind="ExternalOutput")

    with (
        nc.sbuf_tensor("ab_t", [K, QPC + nb * C], bf16) as ab_t,
        nc.sbuf_tensor("mx_all", [P, TILES * 8], f32) as mx_all,
        nc.psum_tensor("ps", [P, PSUM_WORDS], f32) as ps,
        nc.semaphore("dma_in") as dma_in,
        nc.semaphore("dma_in2") as dma_in2,
        nc.semaphore("dma_in3") as dma_in3,
        nc.semaphore("pe_sem") as pe_sem,
        nc.semaphore("dve_sem") as dve_sem,
        nc.semaphore("dma_out") as dma_out,
    ):
        if keyed:
            _emit(nc, C, nb, slot_words, n_slots, AB, OUT, ab_t, mx_all,
                  None, ps, dma_in, dma_in2, dma_in3, pe_sem, dve_sem,
                  dma_out, keyed=True)
        else:
            with nc.sbuf_tensor("ix_all", [P, TILES * 8], u32) as ix_all:
                _emit(nc, C, nb, slot_words, n_slots, AB, OUT, ab_t, mx_all,
                      ix_all, ps, dma_in, dma_in2, dma_in3, pe_sem, dve_sem,
                      dma_out, keyed=False)
    return nc


def _emit(nc, C, nb, slot_words, n_slots, AB, OUT, ab_t, mx_all, ix_all,
          ps, dma_in, dma_in2, dma_in3, pe_sem, dve_sem, dma_out, keyed):
    dve_per_tile = 1 if keyed else 2
    out_sb = mx_all if keyed else ix_all
    # column layout: nb>1 -> [B0 | A | B1..B11] so one small DMA covers
    # everything tile 0 needs; nb==1 -> [A | B].
    a_off = C if nb > 1 else 0
    cut = C + QPC if nb > 1 else QPC + C          # first-DMA extent
    nb_mid = (nb - 1) // 2                        # blocks 1..nb_mid on ACT
    cut2 = cut + nb_mid * C

    def boff(i):
        if nb == 1:
            return QPC
        return 0 if i == 0 else C + QPC + (i - 1) * C

    n_out = 4                                     # output DMA chunks
    step = TILES // n_out

    with nc.Block() as block:

        if nb > 1:

            @block.scalar
            def _(scalar):
                # parallel HWDGE queue for the next candidate blocks
                scalar.dma_start(ab_t[:, cut:cut2], AB[:, cut:cut2]) \
                    .then_inc(dma_in2, 16)

        @block.sync
        def _(sync):
            sync.dma_start(ab_t[:, :cut], AB[:, :cut]).then_inc(dma_in, 16)
            if nb > 1:
                sync.dma_start(ab_t[:, cut2:], AB[:, cut2:]).then_inc(
                    dma_in3, 16)
            # ship results in chunks so the final DMA is small; no final
            # wait on dma_out - the module epilogue drains the DMA queues,
            # overlapping the last chunk's completion latency with the
            # semaphore-reset storm.
            for k in range(n_out):
                sync.wait_ge(dve_sem, dve_per_tile * step * (k + 1))
                sl = slice(step * 8 * k, step * 8 * (k + 1))
                sync.dma_start(OUT[:, sl], out_sb[:, sl]).then_inc(
                    dma_out, 16)

        @block.tensor
        def _(tensor):
            # probe: timestamps when the PE body becomes executable
            tensor.wait_ge(dve_sem, 0)
            tensor.wait_ge(dma_in, 16)
            for i in range(TILES):
                if i == 1 and nb > 1:
                    tensor.wait_ge(dma_in2, 16)
                if i == 1 + nb_mid and nb > 1:
                    tensor.wait_ge(dma_in3, 16)
                if i >= n_slots:
                    # PSUM slot (i % n_slots) must be drained by the DVE
                    tensor.wait_ge(
                        dve_sem, dve_per_tile * (i - n_slots + 1))
                base = (i % n_slots) * slot_words
                lhsT = ab_t[:, a_off + i * P:a_off + (i + 1) * P]
                last = None
                for j0 in range(0, C, 512):
                    j1 = min(j0 + 512, C)
                    last = tensor.matmul(
                        ps[:, base + j0:base + j1],
                        lhsT,
                        ab_t[:, boff(i) + j0:boff(i) + j1],
                    )
                last.then_inc(pe_sem)

        @block.vector
        def _(vector):
            for i in range(TILES):
                vector.wait_ge(pe_sem, i + 1)
                base = (i % n_slots) * slot_words
                sc_i = ps[:, base:base + C]
                vector.max(
                    out=mx_all[:, i * 8:(i + 1) * 8], in_=sc_i
                ).then_inc(dve_sem)
                if not keyed:
                    vector.wait_ge(dve_sem, 2 * i + 1)
                    vector.max_index(
                        out=ix_all[:, i * 8:(i + 1) * 8],
                        in_max=mx_all[:, i * 8:(i + 1) * 8],
                        in_values=sc_i,
                    ).then_inc(dve_sem)


def _get_module(C, keyed, per_tile_b):
    key = (C, keyed, per_tile_b)
    if key not in _module_cache:
        _module_cache[key] = _build_module(C, keyed, per_tile_b)
    return _module_cache[key]


def _run_device(ABmat, C, keyed, per_tile_b):
    """ABmat: list of [K, cols] bf16 per core -> (N, 8) out values."""
    from concourse.bass_utils import run_bass_kernel_spmd

    nc = _get_module(C, keyed, per_tile_b)
    in_maps = [{"AB": ab} for ab in ABmat]
    res = run_bass_kernel_spmd(nc, in_maps, core_ids=list(range(N_CORES)))
    global LAST_RESULTS
    LAST_RESULTS = res
    outs = []
    for r in res.results:
        o = r["OUT"].reshape(P, TILES, 8)          # [p, tile, rank]
        outs.append(o.transpose(1, 0, 2).reshape(QPC, 8))
    return np.concatenate(outs, axis=0)


def _query_features(keyed):
    """Per-query lhsT rows [K, N] as float (bf16-exact integer values)."""
    q = np.arange(N)
    qx = (q % W).astype(np.float64)
    qy = (q // W).astype(np.float64)
    if keyed:
        # 2*KEY_M*qx = 1280*qx split as 20480*(qx>>4) + 1280*(qx&15).
        # K order chosen so partial sums stay exact fp32 integers under
        # either PE accumulation direction.
        rows = [
            20480.0 * np.floor(qx / 16),   # * cx
            np.full(N, -65536.0),          # * v2
            20480.0 * np.floor(qy / 16),   # * cy
            1280.0 * (qx % 16),            # * cx
            1280.0 * (qy % 16),            # * cy
            np.full(N, -256.0),            # * v1
            np.full(N, -1.0),              # * v0
        ]
    else:
        rows = [
            2.0 * qx,                      # * cx
            2.0 * qy,                      # * cy
            np.full(N, -256.0),            # * w1
            np.full(N, -1.0),              # * w0
        ]
    return np.stack(rows)                  # [K, N]


def _cand_features(keyed, C, cand_idx):
    """Per-candidate rhs rows [K, C] incl. padding columns.

    cand_idx: pixel indices of this block's candidates (ascending)."""
    n = cand_idx.size
    cx = (cand_idx % W).astype(np.float64)
    cy = (cand_idx // W).astype(np.float64)
    Bm = np.zeros((7 if keyed else 4, C), np.float64)
    if keyed:
        v = KEY_M * (cx * cx + cy * cy) + np.arange(n, dtype=np.float64)
        assert n <= KEY_M and v.max(initial=0) < 2 ** 24
        Bm[0, :n] = cx
        Bm[1, :n] = np.floor(v / 65536)
        Bm[2, :n] = cy
        Bm[3, :n] = cx
        Bm[4, :n] = cy
        Bm[5, :n] = np.floor(v / 256) % 256
        Bm[6, :n] = v % 256
        # padding: key = -(65536+256+1)*255 = -16777215 < any real key
        Bm[1, n:] = 255.0
        Bm[5, n:] = 255.0
        Bm[6, n:] = 255.0
    else:
        w = cx * cx + cy * cy              # <= 25154
        Bm[0, :n] = cx
        Bm[1, :n] = cy
        Bm[2, :n] = np.floor(w / 256)
        Bm[3, :n] = w % 256
        # padding: score = -65535 < real score min (-25154)
        Bm[2, n:] = 255.0
        Bm[3, n:] = 255.0
    return Bm


def _row_radius(valid_idx):
    """Per pixel row y0: radius R such that every query in row y0 has its
    4 nearest valid pixels within |cy - y0| <= R. Bound: exact 4th-NN
    distance at every 2x2-cell center plus the cell radius (triangle
    inequality with the center's four nearest as witnesses)."""
    cx = (valid_idx % W).astype(np.float64)
    cy = (valid_idx // W).astype(np.float64)
    ccx = np.arange(W // 2) * 2 + 0.5
    ccy = np.arange(H // 2) * 2 + 0.5
    dx = ccx[None, :, None] - cx[None, None, :]
    dy = ccy[:, None, None] - cy[None, None, :]
    d = np.sqrt(dx * dx + dy * dy)                # [H/2, W/2, n_valid]
    d4 = np.partition(d, N_NEIGHBORS - 1, axis=2)[:, :, N_NEIGHBORS - 1]
    bound = d4.max(axis=1) + np.sqrt(0.5)         # per cell row
    return np.ceil(bound[np.arange(H) // 2]).astype(np.int64)


def _host_fallback(flat, valid_idx):
    """Exact numpy replication of the reference for degenerate inputs."""
    q = np.arange(N)
    qx = (q % W).astype(np.float32)
    qy = (q // W).astype(np.float32)
    cx = (valid_idx % W).astype(np.float32)
    cy = (valid_idx // W).astype(np.float32)
    pos4 = np.empty((N, N_NEIGHBORS), np.int64)
    chunk = 512
    for s in range(0, N, chunk):
        e = min(s + chunk, N)
        dx = qx[s:e, None] - cx[None, :]
        dy = qy[s:e, None] - cy[None, :]
        sc = np.full((e - s, N), -np.inf, np.float32)
        sc[:, valid_idx] = -(dx * dx + dy * dy)
        order = np.argsort(-sc, axis=1, kind="stable")
        pos4[s:e] = order[:, :N_NEIGHBORS]
    return pos4  # already pixel indices (full-N score rows)


def _pack_ab(Arows, Bblocks, C):
    """Assemble per-core AB matrices. Bblocks: [N_CORES][nb] of [K, C].
    nb>1 layout: [B0 | A | B1..B11]; nb==1 layout: [A | B]."""
    import ml_dtypes

    Kdim = Arows.shape[0]
    ABmat = []
    for c in range(N_CORES):
        nb = len(Bblocks[c])
        ab = np.empty((Kdim, QPC + nb * C), np.float64)
        if nb == 1:
            ab[:, :QPC] = Arows[:, c * QPC:(c + 1) * QPC]
            ab[:, QPC:] = Bblocks[c][0]
        else:
            ab[:, :C] = Bblocks[c][0]
            ab[:, C:C + QPC] = Arows[:, c * QPC:(c + 1) * QPC]
            for i in range(1, nb):
                ab[:, C + QPC + (i - 1) * C:C + QPC + i * C] = Bblocks[c][i]
        ABmat.append(np.ascontiguousarray(ab.astype(ml_dtypes.bfloat16)))
    return ABmat


def kernel(S):
    S = np.asarray(S)
    flat = S.reshape(-1).astype(np.float32)
    valid_idx = np.flatnonzero(flat > V_THRESH)
    n_valid = int(valid_idx.size)

    if n_valid < 8 or n_valid > MAX_C:
        args_nq = _host_fallback(flat, valid_idx)
    else:
        cy = valid_idx // W
        R = _row_radius(valid_idx)                         # (H,)
        bands = [valid_idx[(cy >= y0 - R[y0]) & (cy <= y0 + R[y0])]
                 for y0 in range(H)]
        c_band = max(b.size for b in bands)
        C_tile = max(64, ((c_band + 31) // 32) * 32)

        if C_tile <= MAX_C_KEYED and C_tile < n_valid:
            # banded keyed path: per-tile candidate y-bands
            Arows = _query_features(True)
            Bblocks = [[_cand_features(True, C_tile, bands[12 * c + i])
                        for i in range(TILES)] for c in range(N_CORES)]
            out = _run_device(_pack_ab(Arows, Bblocks, C_tile),
                              C_tile, True, True)
            keys = out[:, :N_NEIGHBORS].astype(np.float64)
            pos = np.mod(-keys, KEY_M).astype(np.int64)    # (N, 4) local
            band_arr = np.zeros((H, C_tile), np.int64)
            for y0 in range(H):
                band_arr[y0, :bands[y0].size] = bands[y0]
            args_nq = band_arr[(np.arange(N) // W)[:, None], pos]
        else:
            # shared full candidate set
            C = max(P, ((n_valid + P - 1) // P) * P)
            keyed = C <= MAX_C_KEYED
            Arows = _query_features(keyed)
            Bblock = _cand_features(keyed, C, valid_idx)
            out = _run_device(_pack_ab(Arows, [[Bblock]] * N_CORES, C),
                              C, keyed, False)
            if keyed:
                keys = out[:, :N_NEIGHBORS].astype(np.float64)
                pos = np.mod(-keys, KEY_M).astype(np.int64)
            else:
                pos = out[:, :N_NEIGHBORS].astype(np.int64)
            args_nq = valid_idx[pos]

    args = args_nq.T.astype(np.int32)[None]                # (1, 4, N)
    ipc = np.empty((1, 2, N_NEIGHBORS, N), np.float32)
    ipc[0, 0] = (args[0] % W).astype(np.float32)
    ipc[0, 1] = (args[0] // W).astype(np.float32)
    return ipc, args
